# revision 1
# baseline (speedup 1.0000x reference)
"""Trainium2 Bass kernel for nn_MDFO (CNL + PNL non-local blocks + CBAM + fusion).

Strategy (pure data-parallel, B=8 over 8 cores, params replicated):
  - Each core processes one sample in channel-major [C, N] layout, N = H*W = 4096.
  - PNL affinity reassociated: (t2 @ p2) @ g2 == t2 @ (p2 @ g2) -> [128,128]
    instead of [2048,2048]; channel-folding handled by a block permutation.
  - BN + all biases folded on host; 1/Cl, 1/M, 1/256, fusion weight folded
    into weights; cnl BN bias absorbed into downstream linear consumers.
  - float32r (full-rate fp32 matmul) everywhere, bf16 for the transposed
    affinity operands (thT, phT, PT, GT).
Validated against the reference in numpy to 2.3e-5 rel l2 (bf16 path).
"""
import sys

import numpy as np

sys.path.insert(0, "/opt/trn_rl_repo")

import ml_dtypes  # noqa: E402

import concourse.bass as bass  # noqa: E402
import concourse.bacc as bacc  # noqa: E402
import concourse.tile as tile  # noqa: E402
from concourse import mybir  # noqa: E402
from concourse.bass_utils import run_bass_kernel_spmd  # noqa: E402

EPS = 1e-5
F32 = mybir.dt.float32
F32R = mybir.dt.float32r
BF16 = mybir.dt.bfloat16
AF = mybir.ActivationFunctionType
ALU = mybir.AluOpType

Ch, Cl, H, W = 256, 128, 64, 64
N = H * W            # 4096
M = N // 2           # 2048
r = Cl // 2          # 64
NEG = -3.0e38


def _R(ap):
    return ap.bitcast(F32R)


def fold_params(inp):
    """Host-side constant folding (all O(param-size) numpy)."""
    f = {}
    scale1 = inp['cnl_bn_g'] / np.sqrt(inp['cnl_bn_v'] + EPS)
    cnl_bf = (inp['cnl_W_b'] * scale1 + inp['cnl_bn_b']
              - inp['cnl_bn_m'] * scale1).astype(np.float32)
    scale2 = inp['pnl_bn_g'] / np.sqrt(inp['pnl_bn_v'] + EPS)
    pnl_bf = (inp['pnl_W_b'] * scale2 + inp['pnl_bn_b']
              - inp['pnl_bn_m'] * scale2).astype(np.float32)
    w_fuse = float(inp['fusion_weight'])

    # step-1 combined x0 conv rhs [Cl, 256]: [phi_wT | phi2_wT | g2_wT']
    f['w_x0cat'] = np.concatenate([
        inp['cnl_phi_w'].T,
        inp['pnl_phi_w'].T,
        (inp['pnl_g_w'] / M).T,
    ], axis=1).astype(np.float32)                                   # [128, 256]
    brow = np.concatenate([
        inp['cnl_phi_b'], inp['pnl_phi_b'], inp['pnl_g_b'] / M,
    ]).astype(np.float32)
    f['b_x0cat'] = np.tile(brow[None, :], (128, 1)).astype(np.float32)  # [128, 256]

    # thT: bf16 weights, two K-chunks side by side [128, 256]
    thT = inp['cnl_theta_w'].T.astype(np.float32)                   # [256, 128]
    f['w_th_bf'] = np.concatenate([thT[:128], thT[128:]], axis=1).astype(
        ml_dtypes.bfloat16)                                         # [128, 256] bf16
    f['b_th'] = np.tile(inp['cnl_theta_b'][None, :], (128, 1)).astype(np.float32)

    f['w_g'] = (inp['cnl_g_w'] / Cl).T.astype(np.float32)           # [128, 128]
    f['b_g'] = (inp['cnl_g_b'] / Cl).astype(np.float32)[:, None]    # [128, 1]

    f['w_cnlW'] = (scale1[:, None] * inp['cnl_W_w']).T.astype(np.float32)  # [128, 256]

    th2 = inp['pnl_theta_w'].T.astype(np.float32)                   # [256, 64]
    f['w_th2'] = np.concatenate([th2[:128], th2[128:]], axis=1).astype(np.float32)  # [128, 128]
    b_th2 = (inp['pnl_theta_b'] + inp['pnl_theta_w'] @ cnl_bf).astype(np.float32)
    f['b_th2'] = b_th2[:, None]                                     # [64, 1]

    f['w_pnlW'] = (scale2[:, None] * inp['pnl_W_w']).T.astype(np.float32)  # [64, 256]
    Wf = (scale1[:, None] * inp['cnl_W_w']).astype(np.float32)      # [256, 128]
    f['w_tyT'] = (inp['pnl_theta_w'] @ Wf).T.astype(np.float32)     # [128, 64]
    bias2 = (pnl_bf + cnl_bf).astype(np.float32)                    # [256]
    f['b2'] = np.stack([bias2[:128], bias2[128:]], axis=1).astype(np.float32)  # [128, 2]

    fc1 = inp['ca_fc1_w'].T.astype(np.float32)                      # [256, 16]
    f['fc1T'] = np.concatenate([fc1[:128], fc1[128:]], axis=1).astype(np.float32)  # [128, 32]
    f['fc2T'] = inp['ca_fc2_w'].T.astype(np.float32)                # [16, 256]

    # sa conv: banded x-conv mats; 1/256 and 1/w_fuse folds
    sa_w = np.asarray(inp['sa_conv_w'][0], dtype=np.float32).copy() # [2, 7, 7]
    sa_w[0] /= (256.0 * w_fuse)
    sa_w[1] /= w_fuse
    Kcat = np.zeros((2, 64, 7 * 64), dtype=np.float32)
    for ch in range(2):
        for dy in range(7):
            for dx in range(7):
                w_ = sa_w[ch, dy, dx]
                if w_ == 0.0:
                    continue
                for x in range(64):
                    xp = x + dx - 3
                    if 0 <= xp < 64:
                        Kcat[ch, xp, dy * 64 + x] = w_
    f['Kcat2'] = np.concatenate([Kcat[0], Kcat[1]], axis=1).astype(np.float32)  # [64, 896]
    Sdy = np.zeros((64, 7 * 64), dtype=np.float32)
    for dy in range(7):
        for y in range(64):
            yp = y + dy - 3
            if 0 <= yp < 64:
                Sdy[yp, dy * 64 + y] = 1.0
    perm = np.array([2 * (q % 32) + q // 32 for q in range(64)])
    f['Sdy7'] = Sdy[perm, :].copy()                                 # [64, 448] pi-rows

    f['b2row'] = bias2[None, :].astype(np.float32)                  # [1, 256]
    f['b_x0row'] = brow[None, :].astype(np.float32)                 # [1, 256]
    f['ones512'] = np.ones((1, 512), dtype=np.float32)
    f['ident'] = np.eye(128, dtype=np.float32)
    f['ones1'] = np.ones((1, 128), dtype=np.float32)
    f['w_fuse'] = w_fuse
    return f


CONST_SPECS = [
    # name, shape, dtype
    ('w_x0cat', [128, 256], F32R),
    ('b_x0cat', [128, 256], F32),
    ('w_th_bf', [128, 256], BF16),
    ('b_th', [128, 128], F32),
    ('w_g', [128, 128], F32R),
    ('b_g', [128, 1], F32),
    ('w_cnlW', [128, 256], F32R),
    ('w_th2', [128, 128], F32R),
    ('b_th2', [64, 1], F32),
    ('w_pnlW', [64, 256], F32R),
    ('w_tyT', [128, 64], F32R),
    ('b2', [128, 2], F32),
    ('fc1T', [128, 32], F32R),
    ('fc2T', [16, 256], F32R),
    ('Kcat2', [64, 896], BF16),
    ('Sdy7', [64, 448], BF16),
    ('ident', [128, 128], F32R),
    ('ones1', [1, 128], F32R),
]


def build_nc(w_fuse):
    nc = bacc.Bacc(None)
    x_d = nc.declare_dram_parameter("x", [256, N], F32R, isOutput=False)
    x0_d = nc.declare_dram_parameter("x0", [128, N], F32R, isOutput=False)
    cd = {}
    for name, shape, dt_ in CONST_SPECS:
        cd[name] = nc.declare_dram_parameter(name, shape, dt_, isOutput=False)
    out_d = nc.declare_dram_parameter("out", [256, N], F32, isOutput=True)
    smean_d = nc.dram_tensor("smean", [1, N], F32R)
    ssig_d = nc.dram_tensor("ssig", [1, N], F32R)

    with tile.TileContext(nc) as tc:
        _frees = []

        def _keep(pair):
            _frees.append(pair[1])
            return pair[0]

        # ---- persistent SBUF tensors ----
        x_t = _keep(tc.tile([128, 2, N], F32R, name="x_t"))
        x0_t = _keep(tc.tile([128, N], F32R, name="x0_t"))
        x0cat = _keep(tc.tile([128, 32, 256], BF16, name="x0cat"))
        thT = _keep(tc.tile([128, 32, 128], BF16, name="thT"))
        gxy = _keep(tc.tile([128, N], F32R, name="gxy"))  # g_x, later overwritten by y
        z_t = _keep(tc.tile([128, 2, N], F32R, name="z_t"))  # z_cnl, later zp in-place
        T_t = _keep(tc.tile([64, N], F32R, name="T_t"))
        Y_t = _keep(tc.tile([64, N], F32R, name="Y_t"))
        attT = _keep(tc.tile([128, 128], F32R, name="attT"))
        S_t = _keep(tc.tile([64, 256], F32R, name="S_t"))
        tA = _keep(tc.tile([128, N], BF16, name="tA"))
        mapT_mean = _keep(tc.tile([64, 64], BF16, name="mapT_mean"))
        PM = _keep(tc.tile([128, 32], BF16, name="PM"))
        PModd = _keep(tc.tile([64, 32], BF16, name="PModd"))
        row_t = _keep(tc.tile([1, N], F32R, name="row_t"))
        meanrow = row_t
        sigrow = row_t
        sig2d = _keep(tc.tile([64, 64], F32R, name="sig2d"))
        mean2d = _keep(tc.tile([64, 64], F32R, name="mean2d"))
        psum_cols = _keep(tc.tile([128, 2, 8], F32, name="psum_cols"))  # sum(Wy+b) per tile
        maxcols = _keep(tc.tile([128, 2, 8], F32, name="maxcols"))
        V_t = _keep(tc.tile([128, 2, 2], F32R, name="V_t"))  # [p, chunk, (mean,max)]
        h_t = _keep(tc.tile([16, 2], F32R, name="h_t"))
        ca_t = _keep(tc.tile([128, 2], F32R, name="ca_t"))  # ca' per chunk
        tmp1 = _keep(tc.tile([128, 4], F32, name="tmp1"))

        from contextlib import ExitStack
        stack = ExitStack()
        # ---- DMA order: x0 chunks, stage-A consts, x chunks, rest ----
        cpool = stack.enter_context(tc.tile_pool(name="consts", bufs=1))
        consts = {}
        for name, shape, dt_ in CONST_SPECS:
            consts[name] = cpool.tile(shape, dt_, name=f"c_{name}", tag=f"c_{name}")
        EARLY = ['w_x0cat', 'b_x0cat', 'w_th_bf', 'b_th', 'w_g', 'b_g']
        for q in range(4):
            nc.sync.dma_start(out=x0_t[:, bass.ts(q, 1024)],
                              in_=x0_d[:, bass.ts(q, 1024)])
        for name in EARLY:
            nc.sync.dma_start(out=consts[name][:, :], in_=cd[name][:, :])
        for q in range(4):
            nc.sync.dma_start(out=x_t[:, 0, bass.ts(q, 1024)],
                              in_=x_d[0:128, bass.ts(q, 1024)])
            nc.sync.dma_start(out=x_t[:, 1, bass.ts(q, 1024)],
                              in_=x_d[128:256, bass.ts(q, 1024)])
        for name, shape, dt_ in CONST_SPECS:
            if name not in EARLY:
                nc.sync.dma_start(out=consts[name][:, :], in_=cd[name][:, :])

        ps = stack.enter_context(tc.tile_pool(name="ps", bufs=7, space="PSUM"))
        ps1 = stack.enter_context(tc.tile_pool(name="ps1", bufs=1, space="PSUM"))
        sp = stack.enter_context(tc.tile_pool(name="sp", bufs=3))

        c = consts
        # =========== Stage A: step1 conv + thT + g_x + att ===========
        att_ps = ps1.tile([128, 128], F32, tag="att", name="att_ps")
        for t in range(8):
            # cast x chunk to bf16
            xbf = sp.tile([128, 2, 512], BF16, tag="xbf", name="xbf")
            nc.scalar.activation(out=xbf[:, 0, :], in_=x_t[:, 0, bass.ts(t, 512)],
                                 func=AF.Copy)
            nc.scalar.activation(out=xbf[:, 1, :], in_=x_t[:, 1, bass.ts(t, 512)],
                                 func=AF.Copy)
            for sub in range(4):
                i = 4 * t + sub
                # step1 combined conv: [128 n, 256]
                ps_a = ps.tile([128, 256], F32, tag="ps", name="ps_a")
                nc.tensor.matmul(ps_a[:, :], _R(x0_t[:, bass.ts(i, 128)]),
                                 _R(c['w_x0cat'][:, :]), start=True, stop=True)
                nc.vector.tensor_tensor(out=x0cat[:, i, :], in0=ps_a[:, :],
                                        in1=c['b_x0cat'][:, :], op=ALU.add)
                # thT
                ps_b = ps.tile([128, 128], F32, tag="ps", name="ps_b")
                nc.tensor.matmul(ps_b[:, :], xbf[:, 0, bass.ts(sub, 128)],
                                 c['w_th_bf'][:, 0:128], start=True, stop=False)
                nc.tensor.matmul(ps_b[:, :], xbf[:, 1, bass.ts(sub, 128)],
                                 c['w_th_bf'][:, 128:256], start=False, stop=True)
                nc.vector.tensor_tensor(out=thT[:, i, :], in0=ps_b[:, :],
                                        in1=c['b_th'][:, :], op=ALU.add)
                # att accumulation: attT[d, c] += phT_i^T-contract thT_i
                nc.tensor.matmul(att_ps[:, :], x0cat[:, i, 0:128], thT[:, i, :],
                                 start=(i == 0), stop=(i == 31))
            # g_x
            ps_g = ps.tile([128, 512], F32, tag="ps", name="ps_g")
            nc.tensor.matmul(ps_g[:, :], _R(c['w_g'][:, :]),
                             _R(x0_t[:, bass.ts(t, 512)]), start=True, stop=True)
            nc.scalar.activation(out=gxy[:, bass.ts(t, 512)], in_=ps_g[:, :],
                                 func=AF.Identity, bias=c['b_g'][:, :])
        nc.scalar.copy(out=attT[:, :], in_=att_ps[:, :])

        # =========== S blocks: SAL|SAR|SBL|SBR in one [64, 256] psum ===========
        ps_s = ps1.tile([64, 256], F32, tag="att", name="ps_s")
        for j in range(16):
            st = (j == 0)
            sp_ = (j == 15)
            PTa = x0cat[:, j, 128:192]
            PTb = x0cat[:, j + 16, 128:192]
            GTa = x0cat[:, j, 192:256]
            GTb = x0cat[:, j + 16, 192:256]
            nc.tensor.matmul(ps_s[:, 0:64], PTa, GTa, start=st, stop=sp_)
            nc.tensor.matmul(ps_s[:, 64:128], PTa, GTb, start=st, stop=sp_)
            nc.tensor.matmul(ps_s[:, 128:192], PTb, GTa, start=st, stop=sp_)
            nc.tensor.matmul(ps_s[:, 192:256], PTb, GTb, start=st, stop=sp_)
        nc.scalar.copy(out=S_t[:, :], in_=ps_s[:, :])


        # =========== y = attT^T-contract g_x  (in-place into gxy) ===========
        for t in range(8):
            ps_y = ps.tile([128, 512], F32, tag="ps", name="ps_y")
            nc.tensor.matmul(ps_y[:, :], _R(attT[:, :]), _R(gxy[:, bass.ts(t, 512)]),
                             start=True, stop=True)
            nc.scalar.copy(out=gxy[:, bass.ts(t, 512)], in_=ps_y[:, :])

        # =========== T = theta2(z) [64, N] ===========
        for t in range(8):
            ps_t = ps.tile([64, 512], F32, tag="ps", name="ps_t")
            nc.tensor.matmul(ps_t[:, :], _R(c['w_tyT'][:, :]),
                             _R(gxy[:, bass.ts(t, 512)]), start=True, stop=False)
            nc.tensor.matmul(ps_t[:, :], _R(c['w_th2'][:, 0:64]),
                             x_t[:, 0, bass.ts(t, 512)], start=False, stop=False)
            nc.tensor.matmul(ps_t[:, :], _R(c['w_th2'][:, 64:128]),
                             x_t[:, 1, bass.ts(t, 512)], start=False, stop=True)
            nc.scalar.activation(out=T_t[:, bass.ts(t, 512)], in_=ps_t[:, :],
                                 func=AF.Identity, bias=c['b_th2'][:, :])

        # =========== Y [64, N] ===========
        for h in range(2):
            for t in range(4):
                ps_Y = ps.tile([64, 512], F32, tag="ps", name="ps_Y")
                nc.tensor.matmul(ps_Y[:, :], _R(S_t[:, h * 64:h * 64 + 64]),
                                 _R(T_t[:, bass.ts(t, 512)]), start=True, stop=False)
                nc.tensor.matmul(ps_Y[:, :], _R(S_t[:, 128 + h * 64:192 + h * 64]),
                                 _R(T_t[:, M + t * 512:M + (t + 1) * 512]),
                                 start=False, stop=True)
                nc.scalar.copy(out=Y_t[:, h * M + t * 512:h * M + (t + 1) * 512],
                               in_=ps_Y[:, :])

        # =========== z_pnl = pnl_Wf @ Y + bias2 + z  (in-place into z_t) ===========
        for ch in range(2):
            for t in range(8):
                ps_p = ps.tile([128, 512], F32, tag="ps", name="ps_p")
                nc.tensor.matmul(ps_p[:, :], _R(c['w_pnlW'][:, bass.ts(ch, 128)]),
                                 _R(Y_t[:, bass.ts(t, 512)]), start=True, stop=False)
                nc.tensor.matmul(ps_p[:, :], _R(c['w_cnlW'][:, bass.ts(ch, 128)]),
                                 _R(gxy[:, bass.ts(t, 512)]), start=False, stop=True)
                nc.vector.scalar_tensor_tensor(
                    out=z_t[:, ch, bass.ts(t, 512)], in0=ps_p[:, :],
                    scalar=c['b2'][:, ch:ch + 1],
                    in1=x_t[:, ch, bass.ts(t, 512)], op0=ALU.add, op1=ALU.add,
                    accum_out=psum_cols[:, ch, t:t + 1])


        # =========== CBAM channel attention (MLP) ===========
        for ch in range(2):
            nc.vector.reduce_max(out=maxcols[:, ch, 0:1],
                                 in_=z_t[:, ch, :].bitcast(F32),
                                 axis=mybir.AxisListType.X)
            nc.vector.reduce_sum(out=tmp1[:, 2:3], in_=psum_cols[:, ch, :],
                                 axis=mybir.AxisListType.X)
            nc.scalar.activation(out=V_t[:, ch, 0:1], in_=tmp1[:, 2:3],
                                 func=AF.Identity, scale=1.0 / float(N))
            nc.scalar.activation(out=V_t[:, ch, 1:2], in_=maxcols[:, ch, 0:1],
                                 func=AF.Identity)
        ps_f1 = ps.tile([16, 2], F32, tag="ps", name="ps_f1")
        nc.tensor.matmul(ps_f1[:, :], _R(c['fc1T'][:, 0:16]), _R(V_t[:, 0, :]),
                         start=True, stop=False)
        nc.tensor.matmul(ps_f1[:, :], _R(c['fc1T'][:, 16:32]), _R(V_t[:, 1, :]),
                         start=False, stop=True)
        nc.scalar.activation(out=h_t[:, :], in_=ps_f1[:, :], func=AF.Relu)
        for ch in range(2):
            ps_f2 = ps.tile([128, 2], F32, tag="ps", name="ps_f2")
            nc.tensor.matmul(ps_f2[:, :], _R(c['fc2T'][:, bass.ts(ch, 128)]),
                             _R(h_t[:, :]), start=True, stop=True)
            nc.scalar.copy(out=tmp1[:, 2:4], in_=ps_f2[:, :])
            nc.vector.tensor_tensor(out=tmp1[:, 0:1], in0=tmp1[:, 2:3],
                                    in1=tmp1[:, 3:4], op=ALU.add)
            nc.scalar.activation(out=ca_t[:, ch:ch + 1], in_=tmp1[:, 0:1],
                                 func=AF.Sigmoid)
        nc.vector.tensor_scalar_mul(ca_t[:, :], ca_t[:, :], w_fuse)

        # =========== mean map:  sum_c ca'[c] * zp[c, n] ===========
        for t in range(8):
            ps_m = ps.tile([1, 512], F32, tag="ps", name="ps_m")
            nc.tensor.matmul(ps_m[:, :], _R(ca_t[:, 0:1]),
                             _R(z_t[:, 0, bass.ts(t, 512)]), start=True, stop=False)
            nc.tensor.matmul(ps_m[:, :], _R(ca_t[:, 1:2]),
                             _R(z_t[:, 1, bass.ts(t, 512)]), start=False, stop=True)
            nc.scalar.copy(out=meanrow[:, bass.ts(t, 512)], in_=ps_m[:, :])
        nc.sync.dma_start(out=smean_d[:, :], in_=meanrow[:, :])
        nc.sync.dma_start(out=mean2d[:, :],
                          in_=smean_d[:, :].rearrange("p (y x) -> (p y) x", x=64))
        ps_tm = ps.tile([64, 64], F32R, tag="ps", name="ps_tm")
        nc.tensor.transpose(_R(ps_tm[:, :]), _R(mean2d[:, :]),
                            _R(c['ident'][0:64, 0:64]))
        nc.scalar.copy(out=mapT_mean[:, :], in_=ps_tm[:, :])

        # =========== max map ===========
        identb = sp.tile([128, 128], BF16, tag="identb", name="identb", bufs=1)
        nc.scalar.activation(out=identb[:, :], in_=c['ident'][:, :], func=AF.Copy)
        for q in range(4):
            nc.scalar.activation(out=tA[:, bass.ts(q, 1024)],
                                 in_=z_t[:, 0, bass.ts(q, 1024)], func=AF.Identity,
                                 scale=ca_t[:, 0:1].bitcast(F32))
            nc.vector.scalar_tensor_tensor(out=tA[:, bass.ts(q, 1024)],
                                           in0=z_t[:, 1, bass.ts(q, 1024)],
                                           scalar=ca_t[:, 1:2].bitcast(F32),
                                           in1=tA[:, bass.ts(q, 1024)],
                                           op0=ALU.mult, op1=ALU.max)
        for b4 in range(8):
            ps_tx = ps.tile([128, 4, 128], BF16, tag="ps", name="ps_tx")
            for k in range(4):
                g = 4 * b4 + k
                nc.tensor.transpose(ps_tx[:, k, :], tA[:, bass.ts(g, 128)],
                                    identb[:, :])
            nc.vector.reduce_max(out=PM[:, bass.ts(b4, 4)], in_=ps_tx[:, :, :],
                                 axis=mybir.AxisListType.X)
        nc.sync.dma_start(out=PModd[:, :], in_=PM[64:128, :])

        # =========== sa conv (banded) + sigmoid ===========
        ps_R = ps.tile([64, 448], F32, tag="ps", name="ps_R")
        mapT_meanP = sp.tile([64, 64], BF16, tag="mmp", name="mapT_meanP", bufs=1)
        nc.vector.tensor_copy(
            out=mapT_meanP[:, :],
            in_=mapT_mean[:, :].rearrange("p (c two) -> p two c", two=2))
        nc.tensor.matmul(ps_R[:, :], mapT_meanP[:, :], c['Kcat2'][:, 0:448],
                         start=True, stop=False)
        nc.tensor.matmul(ps_R[0:32, :], PM[0:64, :], c['Kcat2'][:, 448:896],
                         start=False, stop=False)
        nc.tensor.matmul(ps_R[32:64, :], PModd[:, :], c['Kcat2'][:, 448:896],
                         start=False, stop=True, tile_position=(0, 32))
        R_sb = sp.tile([64, 448], BF16, tag="Rsb", name="R_sb")
        nc.scalar.copy(out=R_sb[:, :], in_=ps_R[:, :])
        ps_sa = ps.tile([64, 64], F32, tag="ps", name="ps_sa")
        for dy in range(7):
            nc.tensor.matmul(ps_sa[:, :], c['Sdy7'][:, bass.ts(dy, 64)],
                             R_sb[:, bass.ts(dy, 64)],
                             start=(dy == 0), stop=(dy == 6))
        nc.scalar.activation(out=sig2d[:, :], in_=ps_sa[:, :], func=AF.Sigmoid)
        nc.sync.dma_start(out=ssig_d[:, :].rearrange("p (y x) -> (p y) x", x=64),
                          in_=sig2d[:, :])
        nc.sync.dma_start(out=sigrow[:, :], in_=ssig_d[:, :])

        # =========== final: out = (zp*ca')*sig + (1-w)*x ===========
        for t in range(8):
            ps_bc = ps.tile([128, 512], F32, tag="ps", name="ps_bc")
            nc.tensor.matmul(ps_bc[:, :], _R(c['ones1'][:, :]),
                             _R(sigrow[:, bass.ts(t, 512)]), start=True, stop=True)
            for ch in range(2):
                vt = sp.tile([128, 512], F32, tag="vt", name="vt")
                nc.vector.scalar_tensor_tensor(
                    out=vt[:, :], in0=z_t[:, ch, bass.ts(t, 512)],
                    scalar=ca_t[:, ch:ch + 1].bitcast(F32), in1=ps_bc[:, :],
                    op0=ALU.mult, op1=ALU.mult)
                nc.vector.scalar_tensor_tensor(
                    out=vt[:, :], in0=x_t[:, ch, bass.ts(t, 512)],
                    scalar=1.0 - w_fuse, in1=vt[:, :], op0=ALU.mult, op1=ALU.add)
                nc.sync.dma_start(out=out_d[128 * ch:128 * (ch + 1), bass.ts(t, 512)],
                                  in_=vt[:, :])
        stack.close()
        for fr in reversed(_frees):
            fr()
    nc.compile()
    return nc


_CACHE = {}


def kernel(**inputs):
    inp = {k: np.asarray(v) for k, v in inputs.items()}
    f = fold_params(inp)
    key = round(f['w_fuse'], 9)
    if key not in _CACHE:
        _CACHE[key] = build_nc(f['w_fuse'])
    nc = _CACHE[key]

    B = inp['x'].shape[0]
    in_maps = []
    for b in range(B):
        m = {
            'x': np.ascontiguousarray(inp['x'][b].reshape(256, N).astype(np.float32)),
            'x0': np.ascontiguousarray(inp['x0'][b].reshape(128, N).astype(np.float32)),
        }
        for name, shape, dt_ in CONST_SPECS:
            arr = f[name]
            npdt = ml_dtypes.bfloat16 if dt_ == BF16 else np.float32
            m[name] = np.ascontiguousarray(arr.reshape(shape).astype(npdt))
        in_maps.append(m)

    res = run_bass_kernel_spmd(nc, in_maps, core_ids=list(range(B)))
    out = np.stack([res.results[b]['out'].reshape(256, H, W) for b in range(B)])
    return out.astype(np.float32)



# revision 20
# speedup vs baseline: 1.3971x; 1.3971x over previous
"""Trainium2 Bass kernel for nn_MDFO (CNL + PNL non-local blocks + CBAM + fusion).

Restructured v3 (pure data-parallel, B=8 over 8 cores, params replicated):
  - bf16 inputs (x, x0) uploaded from host; bf16 output, fp32 on host.
  - (1-w)*x residual computed on the otherwise-idle Pool engine.
  - all constants packed into three blob DMAs (early-bf16, late-bf16, f32).
  - y and g_x never materialized: runtime weight folds WA/WC/w_ta/WD with
    rank-1 bias fixups; T2/Y2/S2 stacked layouts halve matmul+copy counts.
  - CBAM mean via matmul accum_out, max via rolling bf16 max accumulator.
  - mean map from raw z with ca as the matmul stationary (starts at ca).
  - w_fuse folded into the sig-broadcast stationary vector.
  - final out = zs*sigb + xp with wide bf16 tensor_tensor ops.
"""
import sys

import numpy as np

sys.path.insert(0, "/opt/trn_rl_repo")

import ml_dtypes  # noqa: E402

import concourse.bass as bass  # noqa: E402
import concourse.bacc as bacc  # noqa: E402
import concourse.tile as tile  # noqa: E402
from concourse import mybir  # noqa: E402
from concourse.bass_utils import run_bass_kernel_spmd  # noqa: E402

EPS = 1e-5
F32 = mybir.dt.float32
F32R = mybir.dt.float32r
BF16 = mybir.dt.bfloat16
AF = mybir.ActivationFunctionType
ALU = mybir.AluOpType

Ch, Cl, H, W = 256, 128, 64, 64
N = H * W            # 4096
M = N // 2           # 2048
r = Cl // 2          # 64

# blob layouts: name -> (col offset, cols, rows)
CBA_COLS = 1152  # early bf16 blob
CBA = {'w_x0cat': (0, 256, 128), 'b_x0cat': (256, 256, 128),
       'w_th_bf': (512, 256, 128), 'b_th_row': (768, 128, 1),
       'ones1': (896, 128, 1), 'ident_bf': (1024, 128, 128)}
CBB_COLS = 1856  # late bf16 blob
CBB = {'w_th2': (0, 128, 128), 'w_pnlW': (128, 256, 128),
       'Kcat2': (384, 896, 64), 'Sdy7': (1280, 448, 64),
       'wones': (1728, 128, 1)}
CF_COLS = 869    # f32 blob
CF = {'ident': (0, 128, 128), 'w_gT': (128, 128, 128), 'b_g': (256, 2, 128),
      'w_cnlW': (258, 256, 128), 'w_tyT': (514, 64, 128),
      'b_th2': (578, 1, 64), 'b2': (579, 2, 128), 'fc1T': (581, 32, 128),
      'fc2T': (613, 256, 16)}


def _R(ap):
    return ap.bitcast(F32R)


def fold_params(inp):
    """Host-side constant folding into three blob arrays."""
    f = {}
    scale1 = inp['cnl_bn_g'] / np.sqrt(inp['cnl_bn_v'] + EPS)
    cnl_bf = (inp['cnl_W_b'] * scale1 + inp['cnl_bn_b']
              - inp['cnl_bn_m'] * scale1).astype(np.float32)
    scale2 = inp['pnl_bn_g'] / np.sqrt(inp['pnl_bn_v'] + EPS)
    pnl_bf = (inp['pnl_W_b'] * scale2 + inp['pnl_bn_b']
              - inp['pnl_bn_m'] * scale2).astype(np.float32)
    w_fuse = float(inp['fusion_weight'])
    f['w_fuse'] = w_fuse

    cbA = np.zeros((128, CBA_COLS), dtype=np.float32)
    cbB = np.zeros((128, CBB_COLS), dtype=np.float32)
    cf = np.zeros((128, CF_COLS), dtype=np.float32)

    def put(blob, table, name, arr):
        off, cols, rows = table[name]
        blob[:rows, off:off + cols] = arr

    put(cbA, CBA, 'w_x0cat', np.concatenate([
        inp['cnl_phi_w'].T, inp['pnl_phi_w'].T, (inp['pnl_g_w'] / M).T],
        axis=1))
    brow = np.concatenate([inp['cnl_phi_b'], inp['pnl_phi_b'],
                           inp['pnl_g_b'] / M])
    put(cbA, CBA, 'b_x0cat', np.tile(brow[None, :], (128, 1)))
    thT = inp['cnl_theta_w'].T
    put(cbA, CBA, 'w_th_bf', np.concatenate([thT[:128], thT[128:]], axis=1))
    put(cbA, CBA, 'b_th_row', inp['cnl_theta_b'][None, :])
    put(cbA, CBA, 'ones1', np.ones((1, 128), dtype=np.float32))
    put(cbA, CBA, 'ident_bf', np.eye(128, dtype=np.float32))

    th2 = inp['pnl_theta_w'].T
    put(cbB, CBB, 'w_th2', np.concatenate([th2[:128], th2[128:]], axis=1))
    w_pnlW = (scale2[:, None] * inp['pnl_W_w']).T
    put(cbB, CBB, 'w_pnlW', np.concatenate([w_pnlW, w_pnlW], axis=0))
    # sa conv banded mats; only 1/256 fold on the mean channel (no w folds)
    sa_w = np.asarray(inp['sa_conv_w'][0], dtype=np.float32).copy()
    sa_w[0] /= 256.0
    Kcat = np.zeros((2, 64, 7 * 64), dtype=np.float32)
    for ch in range(2):
        for dy in range(7):
            for dx in range(7):
                w_ = sa_w[ch, dy, dx]
                if w_ == 0.0:
                    continue
                for x in range(64):
                    xq = x + dx - 3
                    if 0 <= xq < 64:
                        Kcat[ch, xq, dy * 64 + x] = w_
    put(cbB, CBB, 'Kcat2', np.concatenate([Kcat[0], Kcat[1]], axis=1))
    Sdy = np.zeros((64, 7 * 64), dtype=np.float32)
    for dy in range(7):
        for y in range(64):
            yp = y + dy - 3
            if 0 <= yp < 64:
                Sdy[yp, dy * 64 + y] = 1.0
    perm = np.array([2 * (q % 32) + q // 32 for q in range(64)])
    put(cbB, CBB, 'Sdy7', Sdy[perm, :])
    put(cbB, CBB, 'wones', np.full((1, 128), w_fuse, dtype=np.float32))

    put(cf, CF, 'ident', np.eye(128, dtype=np.float32))
    put(cf, CF, 'w_gT', inp['cnl_g_w'] / Cl)
    bgc = (inp['cnl_g_b'] / Cl)[:, None]
    put(cf, CF, 'b_g', np.concatenate([bgc, bgc], axis=1))
    put(cf, CF, 'w_cnlW', (scale1[:, None] * inp['cnl_W_w']).T)
    put(cf, CF, 'w_tyT', (inp['pnl_theta_w'] @ (scale1[:, None] * inp['cnl_W_w'])).T)
    put(cf, CF, 'b_th2', (inp['pnl_theta_b'] + inp['pnl_theta_w'] @ cnl_bf)[:, None])
    bias2 = (pnl_bf + cnl_bf)
    put(cf, CF, 'b2', np.stack([bias2[:128], bias2[128:]], axis=1))
    fc1 = inp['ca_fc1_w'].T
    put(cf, CF, 'fc1T', np.concatenate([fc1[:128], fc1[128:]], axis=1))
    put(cf, CF, 'fc2T', inp['ca_fc2_w'].T)

    f['cbA'] = cbA.astype(ml_dtypes.bfloat16)
    f['cbB'] = cbB.astype(ml_dtypes.bfloat16)
    f['cf'] = cf.astype(np.float32)
    return f


def build_nc(w_fuse):
    nc = bacc.Bacc(None)
    x_d = nc.declare_dram_parameter("x", [128, 2, N], BF16, isOutput=False)
    x0_d = nc.declare_dram_parameter("x0", [128, N], BF16, isOutput=False)
    cbA_d = nc.declare_dram_parameter("cbA", [128, CBA_COLS], BF16, isOutput=False)
    cbB_d = nc.declare_dram_parameter("cbB", [128, CBB_COLS], BF16, isOutput=False)
    cf_d = nc.declare_dram_parameter("cf", [128, CF_COLS], F32R, isOutput=False)
    out_d = nc.declare_dram_parameter("out", [256, N], BF16, isOutput=True)
    smean_d = nc.dram_tensor("smean", [1, N], BF16)
    ssig_d = nc.dram_tensor("ssig", [1, N], BF16)

    with tile.TileContext(nc) as tc:
        _frees = []

        def _keep(pair):
            _frees.append(pair[1])
            return pair[0]

        # ---- persistent SBUF tensors ----
        x_t = _keep(tc.tile([128, 2, N], BF16, name="x_t"))
        xp_t = _keep(tc.tile([128, 2, N], BF16, name="xp_t"))
        x0_t = _keep(tc.tile([128, N], BF16, name="x0_t"))
        cbA_t = _keep(tc.tile([128, CBA_COLS], BF16, name="cbA_t"))
        cbB_t = _keep(tc.tile([128, CBB_COLS], BF16, name="cbB_t"))
        cf_t = _keep(tc.tile([128, CF_COLS], F32R, name="cf_t"))
        x0cat = _keep(tc.tile([128, 32, 256], BF16, name="x0cat"))
        thT = _keep(tc.tile([128, 32, 128], BF16, name="thT"))
        attT = _keep(tc.tile([128, 128], F32R, name="attT"))
        att_s = _keep(tc.tile([128, 128], F32R, name="att_s"))
        WA_s = _keep(tc.tile([128, 256], F32R, name="WA_s"))
        WC_s = _keep(tc.tile([128, 256], BF16, name="WC_s"))
        wta_s = _keep(tc.tile([128, 64], F32R, name="wta_s"))
        WD_s = _keep(tc.tile([128, 64], BF16, name="WD_s"))
        S2_s = _keep(tc.tile([128, 128], BF16, name="S2_s"))
        T2 = _keep(tc.tile([128, M], BF16, name="T2"))
        Y2 = _keep(tc.tile([128, M], BF16, name="Y2"))
        z_t = _keep(tc.tile([128, 2, N], BF16, name="z_t"))
        bz = _keep(tc.tile([128, 2], F32, name="bz"))
        bT2 = _keep(tc.tile([128, 1], F32, name="bT2"))
        psum_cols = _keep(tc.tile([128, 2, 4], F32, name="psum_cols"))
        macc = _keep(tc.tile([128, 2, 512], BF16, name="macc"))
        V_t = _keep(tc.tile([128, 2, 2], F32, name="V_t"))
        h_t = _keep(tc.tile([16, 2], F32, name="h_t"))
        ca_t = _keep(tc.tile([128, 2], F32, name="ca_t"))
        ca_bf = _keep(tc.tile([128, 2], BF16, name="ca_bf"))
        tmp1 = _keep(tc.tile([128, 4], F32, name="tmp1"))
        tA = _keep(tc.tile([128, N], BF16, name="tA"))
        PM = _keep(tc.tile([128, 32], BF16, name="PM"))
        PModd = _keep(tc.tile([64, 32], BF16, name="PModd"))
        m2d_sb = _keep(tc.tile([64, 64], BF16, name="m2d_sb"))
        meanrow = _keep(tc.tile([1, N], BF16, name="meanrow"))
        sigrow = _keep(tc.tile([1, N], BF16, name="sigrow"))
        mapT_mean = _keep(tc.tile([64, 64], BF16, name="mapT_mean"))
        mapT_meanP = _keep(tc.tile([64, 64], BF16, name="mapT_meanP"))
        R_sb = _keep(tc.tile([64, 448], BF16, name="R_sb"))
        sig2d = _keep(tc.tile([64, 64], BF16, name="sig2d"))
        sigb = _keep(tc.tile([128, 1, N], BF16, name="sigb"))

        def cA(name, rows=None):
            off, cols, rws = CBA[name]
            return cbA_t[0:(rows or rws), off:off + cols]

        def cB(name, rows=None):
            off, cols, rws = CBB[name]
            return cbB_t[0:(rows or rws), off:off + cols]

        def cF(name, rows=None):
            off, cols, rws = CF[name]
            return cf_t[0:(rows or rws), off:off + cols]

        from contextlib import ExitStack
        stack = ExitStack()

        # ---- DMAs: first pixel group + early consts, then the rest ----
        nc.sync.dma_start(out=x0_t[:, 0:512], in_=x0_d[:, 0:512])
        nc.sync.dma_start(out=x_t[:, :, 0:512], in_=x_d[:, :, 0:512])
        nc.sync.dma_start(out=cbA_t[:, :], in_=cbA_d[:, :])
        nc.sync.dma_start(out=x0_t[:, 512:2048], in_=x0_d[:, 512:2048])
        nc.sync.dma_start(out=x_t[:, :, 512:2048], in_=x_d[:, :, 512:2048])
        nc.sync.dma_start(out=x0_t[:, 2048:4096], in_=x0_d[:, 2048:4096])
        nc.sync.dma_start(out=x_t[:, :, 2048:4096], in_=x_d[:, :, 2048:4096])
        nc.sync.dma_start(out=cbB_t[:, :], in_=cbB_d[:, :])
        nc.sync.dma_start(out=cf_t[:, :], in_=cf_d[:, :])

        sp = stack.enter_context(tc.tile_pool(name="sp", bufs=3))

        # =========== Stage A: x0cat + thT + att; then S blocks ===========
        with tc.tile_pool(name="psA", bufs=2, space="PSUM") as psA, \
             tc.tile_pool(name="ps1", bufs=1, space="PSUM") as ps1:
            att_ps = ps1.tile([128, 128], F32, tag="att", name="att_ps")
            ps_s = ps1.tile([64, 256], F32, tag="S2", name="ps_s")
            for t8 in range(8):
                ps_x0c = psA.tile([128, 1024], F32, tag="x0c", name="ps_x0c")
                ps_tht = psA.tile([128, 512], F32, tag="tht", name="ps_tht")
                for sub in range(4):
                    i = 4 * t8 + sub
                    nc.tensor.matmul(ps_x0c[:, bass.ts(sub, 256)],
                                     x0_t[:, bass.ts(i, 128)], cA('w_x0cat'),
                                     start=True, stop=True)
                    nc.tensor.matmul(ps_tht[:, bass.ts(sub, 128)],
                                     cA('ones1'), cA('b_th_row'),
                                     start=True, stop=False)
                    nc.tensor.matmul(ps_tht[:, bass.ts(sub, 128)],
                                     x_t[:, 0, bass.ts(i, 128)],
                                     cA('w_th_bf')[:, 0:128],
                                     start=False, stop=False)
                    nc.tensor.matmul(ps_tht[:, bass.ts(sub, 128)],
                                     x_t[:, 1, bass.ts(i, 128)],
                                     cA('w_th_bf')[:, 128:256],
                                     start=False, stop=True)
                nc.vector.tensor_tensor(
                    out=x0cat[:, 4 * t8:4 * t8 + 4, :],
                    in0=ps_x0c[:, :].rearrange("p (a c) -> p a c", c=256),
                    in1=cA('b_x0cat').rearrange("p (a c) -> p a c", c=256
                                                ).broadcast_to([128, 4, 256]),
                    op=ALU.add)
                nc.scalar.activation(
                    out=thT[:, 4 * t8:4 * t8 + 4, :],
                    in_=ps_tht[:, :].rearrange("p (a c) -> p a c", c=128),
                    func=AF.Copy)
                for sub in range(4):
                    i = 4 * t8 + sub
                    nc.tensor.matmul(att_ps[:, :], x0cat[:, i, 0:128],
                                     thT[:, i, :], start=(i == 0), stop=(i == 31))
            # S blocks: consecutive emission (interleaving the four shared-bank
            # psum streams with other matmuls corrupts the accumulation)
            for j in range(16):
                st = (j == 0)
                sp_ = (j == 15)
                PTa = x0cat[:, j, 128:192]
                PTb = x0cat[:, j + 16, 128:192]
                GTa = x0cat[:, j, 192:256]
                GTb = x0cat[:, j + 16, 192:256]
                nc.tensor.matmul(ps_s[:, 0:64], PTa, GTa, start=st, stop=sp_)
                nc.tensor.matmul(ps_s[:, 64:128], PTa, GTb, start=st, stop=sp_)
                nc.tensor.matmul(ps_s[:, 128:192], PTb, GTa, start=st, stop=sp_)
                nc.tensor.matmul(ps_s[:, 192:256], PTb, GTb, start=st, stop=sp_)
            nc.scalar.copy(out=attT[:, :], in_=att_ps[:, :])
            nc.vector.tensor_copy(out=S2_s[0:64, :], in_=ps_s[:, 0:128])
            nc.vector.tensor_copy(out=S2_s[64:128, :], in_=ps_s[:, 128:256])

        # xp = (1-w) * x on the idle Pool engine
        for g in range(4):
            nc.gpsimd.tensor_scalar(out=xp_t[:, :, bass.ts(g, 1024)],
                                    in0=x_t[:, :, bass.ts(g, 1024)],
                                    scalar1=1.0 - w_fuse, scalar2=None,
                                    op0=ALU.mult)

        # =========== folds + T + Y + z + channel attention ===========
        with tc.tile_pool(name="psB", bufs=2, space="PSUM") as psB:
            ps_at = psB.tile([128, 128], F32R, tag="sm", name="ps_at")
            nc.tensor.transpose(_R(ps_at[:, :]), attT[:, :], _R(cF('ident')))
            nc.scalar.copy(out=att_s[:, :], in_=ps_at[:, :])
            ps_wa = psB.tile([128, 256], F32, tag="sm", name="ps_wa")
            nc.tensor.matmul(ps_wa[:, :], att_s[:, :], _R(cF('w_cnlW')),
                             start=True, stop=True)
            nc.scalar.copy(out=WA_s[:, :], in_=ps_wa[:, :])
            ps_wt = psB.tile([128, 64], F32, tag="sm", name="ps_wt")
            nc.tensor.matmul(ps_wt[:, :], att_s[:, :], _R(cF('w_tyT')),
                             start=True, stop=True)
            nc.scalar.copy(out=wta_s[:, :], in_=ps_wt[:, :])
            ps_wc = psB.tile([128, 256], F32, tag="sm", name="ps_wc")
            nc.tensor.matmul(ps_wc[:, :], _R(cF('w_gT')), WA_s[:, :],
                             start=True, stop=True)
            nc.vector.tensor_copy(out=WC_s[:, :], in_=ps_wc[:, :])
            ps_wd = psB.tile([128, 64], F32, tag="sm", name="ps_wd")
            nc.tensor.matmul(ps_wd[:, :], _R(cF('w_gT')), wta_s[:, :],
                             start=True, stop=True)
            nc.vector.tensor_copy(out=WD_s[:, :], in_=ps_wd[:, :])
            ps_bb = psB.tile([128, 4], F32, tag="sm", name="ps_bb")
            nc.tensor.matmul(ps_bb[:, 0:2], WA_s[:, 0:128], _R(cF('b_g')),
                             start=True, stop=True)
            nc.tensor.matmul(ps_bb[:, 2:4], WA_s[:, 128:256], _R(cF('b_g')),
                             start=True, stop=True)
            nc.vector.tensor_tensor(out=bz[:, 0:1], in0=ps_bb[:, 0:1],
                                    in1=cF('b2')[:, 0:1].bitcast(F32), op=ALU.add)
            nc.vector.tensor_tensor(out=bz[:, 1:2], in0=ps_bb[:, 2:3],
                                    in1=cF('b2')[:, 1:2].bitcast(F32), op=ALU.add)
            ps_bt = psB.tile([64, 2], F32, tag="sm", name="ps_bt")
            nc.tensor.matmul(ps_bt[:, :], wta_s[:, :], _R(cF('b_g')),
                             start=True, stop=True)
            nc.vector.tensor_tensor(out=bT2[0:64, :], in0=ps_bt[:, 0:1],
                                    in1=cF('b_th2').bitcast(F32), op=ALU.add)
            nc.vector.tensor_copy(out=bT2[64:128, :], in_=bT2[0:64, :])

            # ---- T2 [128, M] ----
            for tm in range(4):
                ps_T = psB.tile([128, 512], F32, tag="TY", name="ps_T")
                for h in range(2):
                    base = h * M + tm * 512
                    o = ps_T[64 * h:64 * h + 64, :]
                    nc.tensor.matmul(o, WD_s[:, :], x0_t[:, base:base + 512],
                                     start=True, stop=False)
                    nc.tensor.matmul(o, cB('w_th2')[:, 0:64],
                                     x_t[:, 0, base:base + 512],
                                     start=False, stop=False)
                    nc.tensor.matmul(o, cB('w_th2')[:, 64:128],
                                     x_t[:, 1, base:base + 512],
                                     start=False, stop=True)
                nc.scalar.activation(out=T2[:, bass.ts(tm, 512)], in_=ps_T[:, :],
                                     func=AF.Identity, bias=bT2[:, :])

            # ---- Y2 [128, M] ----
            for tm in range(4):
                ps_Y = psB.tile([128, 512], F32, tag="TY", name="ps_Y")
                nc.tensor.matmul(ps_Y[:, :], S2_s[:, :], T2[:, bass.ts(tm, 512)],
                                 start=True, stop=True)
                nc.scalar.activation(out=Y2[:, bass.ts(tm, 512)], in_=ps_Y[:, :],
                                     func=AF.Copy)

            # ---- z [128, 2, N] bf16, paired tiles per psum ----
            for t2 in range(4):
                for ch in range(2):
                    ps_z = psB.tile([128, 1024], F32, tag="z", name="ps_z")
                    for k in range(2):
                        t = 2 * t2 + k
                        h = t // 4
                        mbase = (t % 4) * 512
                        o = ps_z[:, bass.ts(k, 512)]
                        nc.tensor.matmul(o, cB('w_pnlW')[64 * h:64 * h + 64,
                                                         bass.ts(ch, 128)],
                                         Y2[64 * h:64 * h + 64, mbase:mbase + 512],
                                         start=True, stop=False)
                        nc.tensor.matmul(o, WC_s[:, bass.ts(ch, 128)],
                                         x0_t[:, bass.ts(t, 512)],
                                         start=False, stop=(ch == 1))
                        if ch == 0:
                            nc.tensor.matmul(o, cA('ident_bf'),
                                             x_t[:, 0, bass.ts(t, 512)],
                                             start=False, stop=True)
                    if ch == 0:
                        nc.scalar.activation(
                            out=z_t[:, 0, bass.ts(t2, 1024)], in_=ps_z[:, :],
                            func=AF.Identity, bias=bz[:, 0:1],
                            accum_out=psum_cols[:, 0, t2:t2 + 1])
                    else:
                        nc.vector.scalar_tensor_tensor(
                            out=z_t[:, 1, bass.ts(t2, 1024)], in0=ps_z[:, :],
                            scalar=bz[:, 1:2],
                            in1=x_t[:, 1, bass.ts(t2, 1024)], op0=ALU.add,
                            op1=ALU.add, accum_out=psum_cols[:, 1, t2:t2 + 1])
                # rolling channel-wise max accumulator (two 512-steps)
                for k in range(2):
                    t = 2 * t2 + k
                    if t == 0:
                        nc.vector.tensor_copy(out=macc[:, :, :],
                                              in_=z_t[:, :, 0:512])
                    else:
                        nc.vector.tensor_tensor(
                            out=macc[:, :, :], in0=macc[:, :, :],
                            in1=z_t[:, :, bass.ts(t, 512)], op=ALU.max)

            # ---- CBAM channel attention (compressed chain) ----
            nc.vector.reduce_max(out=tmp1[:, 0:2], in_=macc[:, :, :],
                                 axis=mybir.AxisListType.X)
            nc.scalar.activation(out=V_t[:, :, 1:2], in_=tmp1[:, 0:2],
                                 func=AF.Identity)
            nc.vector.reduce_sum(out=tmp1[:, 2:4], in_=psum_cols[:, :, :],
                                 axis=mybir.AxisListType.X)
            nc.scalar.activation(out=V_t[:, :, 0:1], in_=tmp1[:, 2:4],
                                 func=AF.Identity, scale=1.0 / float(N))
            ps_f1 = psB.tile([16, 2], F32, tag="sm", name="ps_f1")
            nc.tensor.matmul(ps_f1[:, :], cF('fc1T')[:, 0:16].bitcast(F32), V_t[:, 0, :],
                             start=True, stop=False)
            nc.tensor.matmul(ps_f1[:, :], cF('fc1T')[:, 16:32].bitcast(F32), V_t[:, 1, :],
                             start=False, stop=True)
            nc.scalar.activation(out=h_t[:, :], in_=ps_f1[:, :], func=AF.Relu)
            for ch in range(2):
                ps_f2 = psB.tile([128, 2], F32, tag="sm", name="ps_f2")
                nc.tensor.matmul(ps_f2[:, :], cF('fc2T')[:, bass.ts(ch, 128)].bitcast(F32),
                                 h_t[:, :], start=True, stop=True)
                nc.vector.reduce_sum(out=tmp1[:, ch:ch + 1], in_=ps_f2[:, :],
                                     axis=mybir.AxisListType.X)
            nc.scalar.activation(out=ca_t[:, :], in_=tmp1[:, 0:2],
                                 func=AF.Sigmoid)
            nc.vector.tensor_copy(out=ca_bf[:, :], in_=ca_t[:, :])

        # =========== maps + sa conv + final ===========
        with tc.tile_pool(name="psC", bufs=2, space="PSUM") as psC:
            # mean map from raw z with ca as stationary (runs right at ca)
            for t in range(8):
                ps_m = psC.tile([1, 512], F32, tag="sm2", name="ps_m")
                nc.tensor.matmul(ps_m[:, :], ca_bf[:, 0:1],
                                 z_t[:, 0, bass.ts(t, 512)],
                                 start=True, stop=False)
                nc.tensor.matmul(ps_m[:, :], ca_bf[:, 1:2],
                                 z_t[:, 1, bass.ts(t, 512)],
                                 start=False, stop=True)
                if t % 2 == 0:
                    nc.vector.tensor_copy(out=meanrow[:, bass.ts(t, 512)],
                                          in_=ps_m[:, :])
                else:
                    nc.scalar.activation(out=meanrow[:, bass.ts(t, 512)],
                                         in_=ps_m[:, :], func=AF.Copy)
            nc.sync.dma_start(out=smean_d[:, :], in_=meanrow[:, :])
            nc.sync.dma_start(
                out=m2d_sb[:, :],
                in_=smean_d[:, :].rearrange("p (a b) -> (p a) b", b=64))
            ps_tm = psC.tile([64, 64], BF16, tag="sm2", name="ps_tm")
            nc.tensor.transpose(ps_tm[:, :], m2d_sb[:, :],
                                cA('ident_bf')[0:64, 0:64])
            nc.vector.tensor_copy(out=mapT_mean[:, :], in_=ps_tm[:, :])
            nc.vector.tensor_copy(
                out=mapT_meanP[:, :],
                in_=mapT_mean[:, :].rearrange("p (c two) -> p two c", two=2))

            # zs = z * ca in place: Act ch0, Pool ch1
            for g in range(2):
                nc.scalar.activation(out=z_t[:, 0, bass.ts(g, 2048)],
                                     in_=z_t[:, 0, bass.ts(g, 2048)],
                                     func=AF.Copy, scale=ca_t[:, 0:1])
                nc.gpsimd.tensor_scalar(out=z_t[:, 1, bass.ts(g, 2048)],
                                        in0=z_t[:, 1, bass.ts(g, 2048)],
                                        scalar1=ca_t[:, 1:2], scalar2=None,
                                        op0=ALU.mult)
            # tA = max over channel chunks
            for g in range(2):
                nc.vector.tensor_tensor(out=tA[:, bass.ts(g, 2048)],
                                        in0=z_t[:, 0, bass.ts(g, 2048)],
                                        in1=z_t[:, 1, bass.ts(g, 2048)],
                                        op=ALU.max)

            # max map: transposes + per-group reduce
            for b4 in range(8):
                ps_tx = psC.tile([128, 4, 128], BF16, tag="tx", name="ps_tx")
                for k in range(4):
                    gidx = 4 * b4 + k
                    nc.tensor.transpose(ps_tx[:, k, :], tA[:, bass.ts(gidx, 128)],
                                        cA('ident_bf'))
                nc.vector.reduce_max(out=PM[:, bass.ts(b4, 4)],
                                     in_=ps_tx[:, :, :],
                                     axis=mybir.AxisListType.X)
            nc.sync.dma_start(out=PModd[:, :], in_=PM[64:128, :])

            # sa conv (banded) + sigmoid
            ps_R = psC.tile([64, 448], F32, tag="sm2", name="ps_R")
            nc.tensor.matmul(ps_R[:, :], mapT_meanP[:, :], cB('Kcat2')[:, 0:448],
                             start=True, stop=False)
            nc.tensor.matmul(ps_R[0:32, :], PM[0:64, :], cB('Kcat2')[:, 448:896],
                             start=False, stop=False)
            nc.tensor.matmul(ps_R[32:64, :], PModd[:, :], cB('Kcat2')[:, 448:896],
                             start=False, stop=True, tile_position=(0, 32))
            nc.vector.tensor_copy(out=R_sb[:, :], in_=ps_R[:, :])
            ps_sa = psC.tile([64, 64], F32, tag="sm2", name="ps_sa")
            for dy in range(7):
                nc.tensor.matmul(ps_sa[:, :], cB('Sdy7')[:, bass.ts(dy, 64)],
                                 R_sb[:, bass.ts(dy, 64)],
                                 start=(dy == 0), stop=(dy == 6))
            nc.scalar.activation(out=sig2d[:, :], in_=ps_sa[:, :], func=AF.Sigmoid)
            nc.sync.dma_start(
                out=ssig_d[:, :].rearrange("p (a b) -> (p a) b", b=64),
                in_=sig2d[:, :])
            nc.sync.dma_start(out=sigrow[:, :], in_=ssig_d[:, :])

            # sig broadcast (w_fuse folded into the stationary ones)
            for t in range(8):
                ps_bc = psC.tile([128, 512], F32, tag="bc", name="ps_bc")
                nc.tensor.matmul(ps_bc[:, :], cB('wones'),
                                 sigrow[:, bass.ts(t, 512)],
                                 start=True, stop=True)
                if t % 2 == 0:
                    nc.scalar.activation(out=sigb[:, 0, bass.ts(t, 512)],
                                         in_=ps_bc[:, :], func=AF.Copy)
                else:
                    nc.vector.tensor_copy(out=sigb[:, 0, bass.ts(t, 512)],
                                          in_=ps_bc[:, :])

            # final: out = zs * sigb + xp (Pool takes group 0's multiply)
            for g in range(4):
                vt = sp.tile([128, 2, 1024], BF16, tag="vt", name="vt")
                sl = bass.ts(g, 1024)
                sgb = sigb[:, :, sl].broadcast_to([128, 2, 1024])
                eng = nc.gpsimd if g == 0 else nc.vector
                eng.tensor_tensor(out=vt[:, :, :], in0=z_t[:, :, sl], in1=sgb,
                                  op=ALU.mult)
                nc.vector.tensor_tensor(out=vt[:, :, :], in0=vt[:, :, :],
                                        in1=xp_t[:, :, sl], op=ALU.add)
                nc.sync.dma_start(
                    out=out_d[:, sl].rearrange("(two p) n -> p two n", two=2),
                    in_=vt[:, :, :])
        stack.close()
        for fr in reversed(_frees):
            fr()
    nc.compile()
    return nc


_CACHE = {}


def kernel(**inputs):
    inp = {k: np.asarray(v) for k, v in inputs.items()}
    f = fold_params(inp)
    key = round(f['w_fuse'], 9)
    if key not in _CACHE:
        _CACHE[key] = build_nc(f['w_fuse'])
    nc = _CACHE[key]

    B = inp['x'].shape[0]
    in_maps = []
    for b in range(B):
        xb = inp['x'][b].reshape(256, N).astype(np.float32)
        m = {
            'x': np.ascontiguousarray(
                xb.reshape(2, 128, N).transpose(1, 0, 2)).astype(ml_dtypes.bfloat16),
            'x0': np.ascontiguousarray(
                inp['x0'][b].reshape(128, N)).astype(ml_dtypes.bfloat16),
            'cbA': f['cbA'], 'cbB': f['cbB'], 'cf': f['cf'],
        }
        in_maps.append(m)

    res = run_bass_kernel_spmd(nc, in_maps, core_ids=list(range(B)))
    out = np.stack([np.asarray(res.results[b]['out'], dtype=np.float32
                               ).reshape(256, H, W) for b in range(B)])
    return out


# revision 21
# speedup vs baseline: 1.4421x; 1.0322x over previous
"""Trainium2 Bass kernel for nn_MDFO (CNL + PNL non-local blocks + CBAM + fusion).

Restructured v3 (pure data-parallel, B=8 over 8 cores, params replicated):
  - bf16 inputs (x, x0) uploaded from host; bf16 output, fp32 on host.
  - (1-w)*x residual computed on the otherwise-idle Pool engine.
  - all constants packed into three blob DMAs (early-bf16, late-bf16, f32).
  - y and g_x never materialized: runtime weight folds WA/WC/w_ta/WD with
    rank-1 bias fixups; T2/Y2/S2 stacked layouts halve matmul+copy counts.
  - CBAM mean via matmul accum_out, max via rolling bf16 max accumulator.
  - mean map from raw z with ca as the matmul stationary (starts at ca).
  - w_fuse folded into the sig-broadcast stationary vector.
  - final out = zs*sigb + xp with wide bf16 tensor_tensor ops.
"""
import sys

import numpy as np

sys.path.insert(0, "/opt/trn_rl_repo")

import ml_dtypes  # noqa: E402

import concourse.bass as bass  # noqa: E402
import concourse.bacc as bacc  # noqa: E402
import concourse.tile as tile  # noqa: E402
from concourse import mybir  # noqa: E402
from concourse.bass_utils import run_bass_kernel_spmd  # noqa: E402

EPS = 1e-5
F32 = mybir.dt.float32
F32R = mybir.dt.float32r
BF16 = mybir.dt.bfloat16
AF = mybir.ActivationFunctionType
ALU = mybir.AluOpType

Ch, Cl, H, W = 256, 128, 64, 64
N = H * W            # 4096
M = N // 2           # 2048
r = Cl // 2          # 64

# blob layouts: name -> (col offset, cols, rows)
CBA_COLS = 1152  # early bf16 blob
CBA = {'w_x0cat': (0, 256, 128), 'b_x0cat': (256, 256, 128),
       'w_th_bf': (512, 256, 128), 'b_th_row': (768, 128, 1),
       'ones1': (896, 128, 1), 'ident_bf': (1024, 128, 128)}
CBB_COLS = 1856  # late bf16 blob
CBB = {'w_th2': (0, 128, 128), 'w_pnlW': (128, 256, 128),
       'Kcat2': (384, 896, 64), 'Sdy7': (1280, 448, 64),
       'wones': (1728, 128, 1)}
CF_COLS = 869    # f32 blob
CF = {'ident': (0, 128, 128), 'w_gT': (128, 128, 128), 'b_g': (256, 2, 128),
      'w_cnlW': (258, 256, 128), 'w_tyT': (514, 64, 128),
      'b_th2': (578, 1, 64), 'b2': (579, 2, 128), 'fc1T': (581, 32, 128),
      'fc2T': (613, 256, 16)}


def _R(ap):
    return ap.bitcast(F32R)


def fold_params(inp):
    """Host-side constant folding into three blob arrays."""
    f = {}
    scale1 = inp['cnl_bn_g'] / np.sqrt(inp['cnl_bn_v'] + EPS)
    cnl_bf = (inp['cnl_W_b'] * scale1 + inp['cnl_bn_b']
              - inp['cnl_bn_m'] * scale1).astype(np.float32)
    scale2 = inp['pnl_bn_g'] / np.sqrt(inp['pnl_bn_v'] + EPS)
    pnl_bf = (inp['pnl_W_b'] * scale2 + inp['pnl_bn_b']
              - inp['pnl_bn_m'] * scale2).astype(np.float32)
    w_fuse = float(inp['fusion_weight'])
    f['w_fuse'] = w_fuse

    cbA = np.zeros((128, CBA_COLS), dtype=np.float32)
    cbB = np.zeros((128, CBB_COLS), dtype=np.float32)
    cf = np.zeros((128, CF_COLS), dtype=np.float32)

    def put(blob, table, name, arr):
        off, cols, rows = table[name]
        blob[:rows, off:off + cols] = arr

    put(cbA, CBA, 'w_x0cat', np.concatenate([
        inp['cnl_phi_w'].T, inp['pnl_phi_w'].T, (inp['pnl_g_w'] / M).T],
        axis=1))
    brow = np.concatenate([inp['cnl_phi_b'], inp['pnl_phi_b'],
                           inp['pnl_g_b'] / M])
    put(cbA, CBA, 'b_x0cat', np.tile(brow[None, :], (128, 1)))
    thT = inp['cnl_theta_w'].T
    put(cbA, CBA, 'w_th_bf', np.concatenate([thT[:128], thT[128:]], axis=1))
    put(cbA, CBA, 'b_th_row', inp['cnl_theta_b'][None, :])
    put(cbA, CBA, 'ones1', np.ones((1, 128), dtype=np.float32))
    put(cbA, CBA, 'ident_bf', np.eye(128, dtype=np.float32))

    th2 = inp['pnl_theta_w'].T
    put(cbB, CBB, 'w_th2', np.concatenate([th2[:128], th2[128:]], axis=1))
    w_pnlW = (scale2[:, None] * inp['pnl_W_w']).T
    put(cbB, CBB, 'w_pnlW', np.concatenate([w_pnlW, w_pnlW], axis=0))
    # sa conv banded mats; only 1/256 fold on the mean channel (no w folds)
    sa_w = np.asarray(inp['sa_conv_w'][0], dtype=np.float32).copy()
    sa_w[0] /= 256.0
    Kcat = np.zeros((2, 64, 7 * 64), dtype=np.float32)
    for ch in range(2):
        for dy in range(7):
            for dx in range(7):
                w_ = sa_w[ch, dy, dx]
                if w_ == 0.0:
                    continue
                for x in range(64):
                    xq = x + dx - 3
                    if 0 <= xq < 64:
                        Kcat[ch, xq, dy * 64 + x] = w_
    put(cbB, CBB, 'Kcat2', np.concatenate([Kcat[0], Kcat[1]], axis=1))
    Sdy = np.zeros((64, 7 * 64), dtype=np.float32)
    for dy in range(7):
        for y in range(64):
            yp = y + dy - 3
            if 0 <= yp < 64:
                Sdy[yp, dy * 64 + y] = 1.0
    perm = np.array([2 * (q % 32) + q // 32 for q in range(64)])
    put(cbB, CBB, 'Sdy7', Sdy[perm, :])
    put(cbB, CBB, 'wones', np.full((1, 128), w_fuse, dtype=np.float32))

    put(cf, CF, 'ident', np.eye(128, dtype=np.float32))
    put(cf, CF, 'w_gT', inp['cnl_g_w'] / Cl)
    bgc = (inp['cnl_g_b'] / Cl)[:, None]
    put(cf, CF, 'b_g', np.concatenate([bgc, bgc], axis=1))
    put(cf, CF, 'w_cnlW', (scale1[:, None] * inp['cnl_W_w']).T)
    put(cf, CF, 'w_tyT', (inp['pnl_theta_w'] @ (scale1[:, None] * inp['cnl_W_w'])).T)
    put(cf, CF, 'b_th2', (inp['pnl_theta_b'] + inp['pnl_theta_w'] @ cnl_bf)[:, None])
    bias2 = (pnl_bf + cnl_bf)
    put(cf, CF, 'b2', np.stack([bias2[:128], bias2[128:]], axis=1))
    fc1 = inp['ca_fc1_w'].T
    put(cf, CF, 'fc1T', np.concatenate([fc1[:128], fc1[128:]], axis=1))
    put(cf, CF, 'fc2T', inp['ca_fc2_w'].T)

    f['cbA'] = cbA.astype(ml_dtypes.bfloat16)
    f['cbB'] = cbB.astype(ml_dtypes.bfloat16)
    f['cf'] = cf.astype(np.float32)
    return f


def build_nc(w_fuse):
    nc = bacc.Bacc(None)
    x_d = nc.declare_dram_parameter("x", [128, 2, N], BF16, isOutput=False)
    x0_d = nc.declare_dram_parameter("x0", [128, N], BF16, isOutput=False)
    cbA_d = nc.declare_dram_parameter("cbA", [128, CBA_COLS], BF16, isOutput=False)
    cbB_d = nc.declare_dram_parameter("cbB", [128, CBB_COLS], BF16, isOutput=False)
    cf_d = nc.declare_dram_parameter("cf", [128, CF_COLS], F32R, isOutput=False)
    out_d = nc.declare_dram_parameter("out", [256, N], BF16, isOutput=True)
    smean_d = nc.dram_tensor("smean", [1, N], BF16)
    ssig_d = nc.dram_tensor("ssig", [1, N], BF16)

    with tile.TileContext(nc) as tc:
        _frees = []

        def _keep(pair):
            _frees.append(pair[1])
            return pair[0]

        # ---- persistent SBUF tensors ----
        x_t = _keep(tc.tile([128, 2, N], BF16, name="x_t"))
        xp_t = _keep(tc.tile([128, 2, N], BF16, name="xp_t"))
        x0_t = _keep(tc.tile([128, N], BF16, name="x0_t"))
        cbA_t = _keep(tc.tile([128, CBA_COLS], BF16, name="cbA_t"))
        cbB_t = _keep(tc.tile([128, CBB_COLS], BF16, name="cbB_t"))
        cf_t = _keep(tc.tile([128, CF_COLS], F32R, name="cf_t"))
        x0cat = _keep(tc.tile([128, 32, 256], BF16, name="x0cat"))
        thT = _keep(tc.tile([128, 32, 128], BF16, name="thT"))
        attT = _keep(tc.tile([128, 128], F32R, name="attT"))
        att_s = _keep(tc.tile([128, 128], F32R, name="att_s"))
        WA_s = _keep(tc.tile([128, 256], F32R, name="WA_s"))
        WC_s = _keep(tc.tile([128, 256], BF16, name="WC_s"))
        wta_s = _keep(tc.tile([128, 64], F32R, name="wta_s"))
        WD_s = _keep(tc.tile([128, 64], BF16, name="WD_s"))
        S2_s = _keep(tc.tile([128, 128], BF16, name="S2_s"))
        T2 = _keep(tc.tile([128, M], BF16, name="T2"))
        Y2 = _keep(tc.tile([128, M], BF16, name="Y2"))
        z_t = _keep(tc.tile([128, 2, N], BF16, name="z_t"))
        bz = _keep(tc.tile([128, 2], F32, name="bz"))
        bT2 = _keep(tc.tile([128, 1], F32, name="bT2"))
        psum_cols = _keep(tc.tile([128, 2, 4], F32, name="psum_cols"))
        macc = _keep(tc.tile([128, 2, 512], BF16, name="macc"))
        V_t = _keep(tc.tile([128, 2, 2], F32, name="V_t"))
        h_t = _keep(tc.tile([16, 2], F32, name="h_t"))
        ca_t = _keep(tc.tile([128, 2], F32, name="ca_t"))
        ca_bf = _keep(tc.tile([128, 2], BF16, name="ca_bf"))
        tmp1 = _keep(tc.tile([128, 4], F32, name="tmp1"))
        tA = _keep(tc.tile([128, N], BF16, name="tA"))
        PM = _keep(tc.tile([128, 32], BF16, name="PM"))
        PModd = _keep(tc.tile([64, 32], BF16, name="PModd"))
        m2d_sb = _keep(tc.tile([64, 64], BF16, name="m2d_sb"))
        meanrow = _keep(tc.tile([1, N], BF16, name="meanrow"))
        sigrow = _keep(tc.tile([1, N], BF16, name="sigrow"))
        mapT_mean = _keep(tc.tile([64, 64], BF16, name="mapT_mean"))
        mapT_meanP = _keep(tc.tile([64, 64], BF16, name="mapT_meanP"))
        R_sb = _keep(tc.tile([64, 448], BF16, name="R_sb"))
        sig2d = _keep(tc.tile([64, 64], BF16, name="sig2d"))
        sigb = _keep(tc.tile([128, 1, N], BF16, name="sigb"))

        def cA(name, rows=None):
            off, cols, rws = CBA[name]
            return cbA_t[0:(rows or rws), off:off + cols]

        def cB(name, rows=None):
            off, cols, rws = CBB[name]
            return cbB_t[0:(rows or rws), off:off + cols]

        def cF(name, rows=None):
            off, cols, rws = CF[name]
            return cf_t[0:(rows or rws), off:off + cols]

        from contextlib import ExitStack
        stack = ExitStack()

        # ---- DMAs: first pixel group + early consts, then the rest ----
        nc.sync.dma_start(out=x0_t[:, 0:512], in_=x0_d[:, 0:512])
        nc.sync.dma_start(out=x_t[:, :, 0:512], in_=x_d[:, :, 0:512])
        nc.sync.dma_start(out=cbA_t[:, :], in_=cbA_d[:, :])
        nc.sync.dma_start(out=x0_t[:, 512:2048], in_=x0_d[:, 512:2048])
        nc.sync.dma_start(out=x_t[:, :, 512:2048], in_=x_d[:, :, 512:2048])
        nc.sync.dma_start(out=x0_t[:, 2048:4096], in_=x0_d[:, 2048:4096])
        nc.sync.dma_start(out=x_t[:, :, 2048:4096], in_=x_d[:, :, 2048:4096])
        nc.sync.dma_start(out=cbB_t[:, :], in_=cbB_d[:, :])
        nc.sync.dma_start(out=cf_t[:, :], in_=cf_d[:, :])

        sp = stack.enter_context(tc.tile_pool(name="sp", bufs=3))

        # warm the sigmoid act-table set (contains identity/copy/relu too)
        warm = sp.tile([1, 8], F32, tag="warm", name="warm", bufs=1)
        nc.vector.memset(warm[:, :], 0.0)
        nc.scalar.activation(out=warm[:, :], in_=warm[:, :], func=AF.Sigmoid)

        # =========== Stage A: x0cat + thT + att; then S blocks ===========
        with tc.tile_pool(name="psA", bufs=2, space="PSUM") as psA, \
             tc.tile_pool(name="ps1", bufs=1, space="PSUM") as ps1:
            att_ps = ps1.tile([128, 128], F32, tag="att", name="att_ps")
            ps_s = ps1.tile([64, 256], F32, tag="S2", name="ps_s")
            for t8 in range(8):
                ps_x0c = psA.tile([128, 1024], F32, tag="x0c", name="ps_x0c")
                ps_tht = psA.tile([128, 512], F32, tag="tht", name="ps_tht")
                for sub in range(4):
                    i = 4 * t8 + sub
                    nc.tensor.matmul(ps_x0c[:, bass.ts(sub, 256)],
                                     x0_t[:, bass.ts(i, 128)], cA('w_x0cat'),
                                     start=True, stop=True)
                    nc.tensor.matmul(ps_tht[:, bass.ts(sub, 128)],
                                     cA('ones1'), cA('b_th_row'),
                                     start=True, stop=False)
                    nc.tensor.matmul(ps_tht[:, bass.ts(sub, 128)],
                                     x_t[:, 0, bass.ts(i, 128)],
                                     cA('w_th_bf')[:, 0:128],
                                     start=False, stop=False)
                    nc.tensor.matmul(ps_tht[:, bass.ts(sub, 128)],
                                     x_t[:, 1, bass.ts(i, 128)],
                                     cA('w_th_bf')[:, 128:256],
                                     start=False, stop=True)
                nc.vector.tensor_tensor(
                    out=x0cat[:, 4 * t8:4 * t8 + 4, :],
                    in0=ps_x0c[:, :].rearrange("p (a c) -> p a c", c=256),
                    in1=cA('b_x0cat').rearrange("p (a c) -> p a c", c=256
                                                ).broadcast_to([128, 4, 256]),
                    op=ALU.add)
                nc.scalar.activation(
                    out=thT[:, 4 * t8:4 * t8 + 4, :],
                    in_=ps_tht[:, :].rearrange("p (a c) -> p a c", c=128),
                    func=AF.Copy)
                for sub in range(4):
                    i = 4 * t8 + sub
                    nc.tensor.matmul(att_ps[:, :], x0cat[:, i, 0:128],
                                     thT[:, i, :], start=(i == 0), stop=(i == 31))
            # S blocks: consecutive emission (interleaving the four shared-bank
            # psum streams with other matmuls corrupts the accumulation)
            for j in range(16):
                st = (j == 0)
                sp_ = (j == 15)
                PTa = x0cat[:, j, 128:192]
                PTb = x0cat[:, j + 16, 128:192]
                GTa = x0cat[:, j, 192:256]
                GTb = x0cat[:, j + 16, 192:256]
                nc.tensor.matmul(ps_s[:, 0:64], PTa, GTa, start=st, stop=sp_)
                nc.tensor.matmul(ps_s[:, 64:128], PTa, GTb, start=st, stop=sp_)
                nc.tensor.matmul(ps_s[:, 128:192], PTb, GTa, start=st, stop=sp_)
                nc.tensor.matmul(ps_s[:, 192:256], PTb, GTb, start=st, stop=sp_)
            nc.scalar.copy(out=attT[:, :], in_=att_ps[:, :])
            nc.vector.tensor_copy(out=S2_s[0:64, :], in_=ps_s[:, 0:128])
            nc.vector.tensor_copy(out=S2_s[64:128, :], in_=ps_s[:, 128:256])

        # xp = (1-w) * x on the idle Pool engine
        for g in range(4):
            nc.gpsimd.tensor_scalar(out=xp_t[:, :, bass.ts(g, 1024)],
                                    in0=x_t[:, :, bass.ts(g, 1024)],
                                    scalar1=1.0 - w_fuse, scalar2=None,
                                    op0=ALU.mult)

        # =========== folds + T + Y + z + channel attention ===========
        with tc.tile_pool(name="psB", bufs=2, space="PSUM") as psB:
            ps_at = psB.tile([128, 128], F32R, tag="sm", name="ps_at")
            nc.tensor.transpose(_R(ps_at[:, :]), attT[:, :], _R(cF('ident')))
            nc.scalar.copy(out=att_s[:, :], in_=ps_at[:, :])
            ps_wa = psB.tile([128, 256], F32, tag="sm", name="ps_wa")
            nc.tensor.matmul(ps_wa[:, :], att_s[:, :], _R(cF('w_cnlW')),
                             start=True, stop=True)
            nc.scalar.copy(out=WA_s[:, :], in_=ps_wa[:, :])
            ps_wt = psB.tile([128, 64], F32, tag="sm", name="ps_wt")
            nc.tensor.matmul(ps_wt[:, :], att_s[:, :], _R(cF('w_tyT')),
                             start=True, stop=True)
            nc.scalar.copy(out=wta_s[:, :], in_=ps_wt[:, :])
            ps_wc = psB.tile([128, 256], F32, tag="sm", name="ps_wc")
            nc.tensor.matmul(ps_wc[:, :], _R(cF('w_gT')), WA_s[:, :],
                             start=True, stop=True)
            nc.vector.tensor_copy(out=WC_s[:, :], in_=ps_wc[:, :])
            ps_wd = psB.tile([128, 64], F32, tag="sm", name="ps_wd")
            nc.tensor.matmul(ps_wd[:, :], _R(cF('w_gT')), wta_s[:, :],
                             start=True, stop=True)
            nc.vector.tensor_copy(out=WD_s[:, :], in_=ps_wd[:, :])
            ps_bb = psB.tile([128, 4], F32, tag="sm", name="ps_bb")
            nc.tensor.matmul(ps_bb[:, 0:2], WA_s[:, 0:128], _R(cF('b_g')),
                             start=True, stop=True)
            nc.tensor.matmul(ps_bb[:, 2:4], WA_s[:, 128:256], _R(cF('b_g')),
                             start=True, stop=True)
            nc.vector.tensor_tensor(out=bz[:, 0:1], in0=ps_bb[:, 0:1],
                                    in1=cF('b2')[:, 0:1].bitcast(F32), op=ALU.add)
            nc.vector.tensor_tensor(out=bz[:, 1:2], in0=ps_bb[:, 2:3],
                                    in1=cF('b2')[:, 1:2].bitcast(F32), op=ALU.add)
            ps_bt = psB.tile([64, 2], F32, tag="sm", name="ps_bt")
            nc.tensor.matmul(ps_bt[:, :], wta_s[:, :], _R(cF('b_g')),
                             start=True, stop=True)
            nc.vector.tensor_tensor(out=bT2[0:64, :], in0=ps_bt[:, 0:1],
                                    in1=cF('b_th2').bitcast(F32), op=ALU.add)
            nc.vector.tensor_copy(out=bT2[64:128, :], in_=bT2[0:64, :])

            # ---- T2 [128, M] ----
            for tm in range(4):
                ps_T = psB.tile([128, 512], F32, tag="TY", name="ps_T")
                for h in range(2):
                    base = h * M + tm * 512
                    o = ps_T[64 * h:64 * h + 64, :]
                    nc.tensor.matmul(o, WD_s[:, :], x0_t[:, base:base + 512],
                                     start=True, stop=False)
                    nc.tensor.matmul(o, cB('w_th2')[:, 0:64],
                                     x_t[:, 0, base:base + 512],
                                     start=False, stop=False)
                    nc.tensor.matmul(o, cB('w_th2')[:, 64:128],
                                     x_t[:, 1, base:base + 512],
                                     start=False, stop=True)
                nc.scalar.activation(out=T2[:, bass.ts(tm, 512)], in_=ps_T[:, :],
                                     func=AF.Identity, bias=bT2[:, :])

            # ---- Y2 [128, M] ----
            for tm in range(4):
                ps_Y = psB.tile([128, 512], F32, tag="TY", name="ps_Y")
                nc.tensor.matmul(ps_Y[:, :], S2_s[:, :], T2[:, bass.ts(tm, 512)],
                                 start=True, stop=True)
                nc.scalar.activation(out=Y2[:, bass.ts(tm, 512)], in_=ps_Y[:, :],
                                     func=AF.Copy)

            # ---- z [128, 2, N] bf16, paired tiles per psum ----
            for t2 in range(4):
                for ch in range(2):
                    ps_z = psB.tile([128, 1024], F32, tag="z", name="ps_z")
                    for k in range(2):
                        t = 2 * t2 + k
                        h = t // 4
                        mbase = (t % 4) * 512
                        o = ps_z[:, bass.ts(k, 512)]
                        nc.tensor.matmul(o, cB('w_pnlW')[64 * h:64 * h + 64,
                                                         bass.ts(ch, 128)],
                                         Y2[64 * h:64 * h + 64, mbase:mbase + 512],
                                         start=True, stop=False)
                        nc.tensor.matmul(o, WC_s[:, bass.ts(ch, 128)],
                                         x0_t[:, bass.ts(t, 512)],
                                         start=False, stop=(ch == 1))
                        if ch == 0:
                            nc.tensor.matmul(o, cA('ident_bf'),
                                             x_t[:, 0, bass.ts(t, 512)],
                                             start=False, stop=True)
                    if ch == 0:
                        nc.scalar.activation(
                            out=z_t[:, 0, bass.ts(t2, 1024)], in_=ps_z[:, :],
                            func=AF.Identity, bias=bz[:, 0:1],
                            accum_out=psum_cols[:, 0, t2:t2 + 1])
                    else:
                        nc.vector.scalar_tensor_tensor(
                            out=z_t[:, 1, bass.ts(t2, 1024)], in0=ps_z[:, :],
                            scalar=bz[:, 1:2],
                            in1=x_t[:, 1, bass.ts(t2, 1024)], op0=ALU.add,
                            op1=ALU.add, accum_out=psum_cols[:, 1, t2:t2 + 1])
                # rolling channel-wise max accumulator (two 512-steps)
                for k in range(2):
                    t = 2 * t2 + k
                    if t == 0:
                        nc.vector.tensor_copy(out=macc[:, :, :],
                                              in_=z_t[:, :, 0:512])
                    else:
                        nc.vector.tensor_tensor(
                            out=macc[:, :, :], in0=macc[:, :, :],
                            in1=z_t[:, :, bass.ts(t, 512)], op=ALU.max)

            # ---- CBAM channel attention (compressed chain) ----
            nc.vector.reduce_max(out=tmp1[:, 0:2], in_=macc[:, :, :],
                                 axis=mybir.AxisListType.X)
            nc.scalar.activation(out=V_t[:, :, 1:2], in_=tmp1[:, 0:2],
                                 func=AF.Identity)
            nc.vector.reduce_sum(out=tmp1[:, 2:4], in_=psum_cols[:, :, :],
                                 axis=mybir.AxisListType.X)
            nc.scalar.activation(out=V_t[:, :, 0:1], in_=tmp1[:, 2:4],
                                 func=AF.Identity, scale=1.0 / float(N))
            ps_f1 = psB.tile([16, 2], F32, tag="sm", name="ps_f1")
            nc.tensor.matmul(ps_f1[:, :], cF('fc1T')[:, 0:16].bitcast(F32), V_t[:, 0, :],
                             start=True, stop=False)
            nc.tensor.matmul(ps_f1[:, :], cF('fc1T')[:, 16:32].bitcast(F32), V_t[:, 1, :],
                             start=False, stop=True)
            nc.scalar.activation(out=h_t[:, :], in_=ps_f1[:, :], func=AF.Relu)
            for ch in range(2):
                ps_f2 = psB.tile([128, 2], F32, tag="sm", name="ps_f2")
                nc.tensor.matmul(ps_f2[:, :], cF('fc2T')[:, bass.ts(ch, 128)].bitcast(F32),
                                 h_t[:, :], start=True, stop=True)
                nc.vector.reduce_sum(out=tmp1[:, ch:ch + 1], in_=ps_f2[:, :],
                                     axis=mybir.AxisListType.X)
            nc.scalar.activation(out=ca_t[:, :], in_=tmp1[:, 0:2],
                                 func=AF.Sigmoid)
            nc.vector.tensor_copy(out=ca_bf[:, :], in_=ca_t[:, :])

        # =========== maps + sa conv + final ===========
        with tc.tile_pool(name="psC", bufs=2, space="PSUM") as psC:
            # mean map from raw z with ca as stationary (runs right at ca)
            for t in range(8):
                ps_m = psC.tile([1, 512], F32, tag="sm2", name="ps_m")
                nc.tensor.matmul(ps_m[:, :], ca_bf[:, 0:1],
                                 z_t[:, 0, bass.ts(t, 512)],
                                 start=True, stop=False)
                nc.tensor.matmul(ps_m[:, :], ca_bf[:, 1:2],
                                 z_t[:, 1, bass.ts(t, 512)],
                                 start=False, stop=True)
                if t % 2 == 0:
                    nc.vector.tensor_copy(out=meanrow[:, bass.ts(t, 512)],
                                          in_=ps_m[:, :])
                else:
                    nc.scalar.activation(out=meanrow[:, bass.ts(t, 512)],
                                         in_=ps_m[:, :], func=AF.Copy)
            nc.scalar.dma_start(out=smean_d[:, :], in_=meanrow[:, :])
            nc.scalar.dma_start(
                out=m2d_sb[:, :],
                in_=smean_d[:, :].rearrange("p (a b) -> (p a) b", b=64))
            ps_tm = psC.tile([64, 64], BF16, tag="sm2", name="ps_tm")
            nc.tensor.transpose(ps_tm[:, :], m2d_sb[:, :],
                                cA('ident_bf')[0:64, 0:64])
            nc.vector.tensor_copy(out=mapT_mean[:, :], in_=ps_tm[:, :])
            nc.vector.tensor_copy(
                out=mapT_meanP[:, :],
                in_=mapT_mean[:, :].rearrange("p (c two) -> p two c", two=2))

            # zs = z * ca in place: Act ch0; ch1 split Pool/DVE
            for g in range(2):
                nc.scalar.activation(out=z_t[:, 0, bass.ts(g, 2048)],
                                     in_=z_t[:, 0, bass.ts(g, 2048)],
                                     func=AF.Copy, scale=ca_t[:, 0:1])
            nc.gpsimd.tensor_scalar(out=z_t[:, 1, 0:2048],
                                    in0=z_t[:, 1, 0:2048],
                                    scalar1=ca_t[:, 1:2], scalar2=None,
                                    op0=ALU.mult)
            nc.vector.tensor_scalar(out=z_t[:, 1, 2048:4096],
                                    in0=z_t[:, 1, 2048:4096],
                                    scalar1=ca_t[:, 1:2], scalar2=None,
                                    op0=ALU.mult)
            # tA = max over channel chunks
            for g in range(2):
                nc.vector.tensor_tensor(out=tA[:, bass.ts(g, 2048)],
                                        in0=z_t[:, 0, bass.ts(g, 2048)],
                                        in1=z_t[:, 1, bass.ts(g, 2048)],
                                        op=ALU.max)

            # max map: transposes + per-group reduce
            for b4 in range(8):
                ps_tx = psC.tile([128, 4, 128], BF16, tag="tx", name="ps_tx")
                for k in range(4):
                    gidx = 4 * b4 + k
                    nc.tensor.transpose(ps_tx[:, k, :], tA[:, bass.ts(gidx, 128)],
                                        cA('ident_bf'))
                nc.vector.reduce_max(out=PM[:, bass.ts(b4, 4)],
                                     in_=ps_tx[:, :, :],
                                     axis=mybir.AxisListType.X)
            nc.sync.dma_start(out=PModd[:, :], in_=PM[64:128, :])

            # sa conv (banded) + sigmoid
            ps_R = psC.tile([64, 448], F32, tag="sm2", name="ps_R")
            nc.tensor.matmul(ps_R[:, :], mapT_meanP[:, :], cB('Kcat2')[:, 0:448],
                             start=True, stop=False)
            nc.tensor.matmul(ps_R[0:32, :], PM[0:64, :], cB('Kcat2')[:, 448:896],
                             start=False, stop=False)
            nc.tensor.matmul(ps_R[32:64, :], PModd[:, :], cB('Kcat2')[:, 448:896],
                             start=False, stop=True, tile_position=(0, 32))
            nc.vector.tensor_copy(out=R_sb[:, :], in_=ps_R[:, :])
            ps_sa = psC.tile([64, 64], F32, tag="sm2", name="ps_sa")
            for dy in range(7):
                nc.tensor.matmul(ps_sa[:, :], cB('Sdy7')[:, bass.ts(dy, 64)],
                                 R_sb[:, bass.ts(dy, 64)],
                                 start=(dy == 0), stop=(dy == 6))
            nc.scalar.activation(out=sig2d[:, :], in_=ps_sa[:, :], func=AF.Sigmoid)
            nc.sync.dma_start(
                out=ssig_d[:, :].rearrange("p (a b) -> (p a) b", b=64),
                in_=sig2d[:, :])
            nc.sync.dma_start(out=sigrow[:, :], in_=ssig_d[:, :])

            # sig broadcast (w_fuse folded into the stationary ones)
            for t in range(8):
                ps_bc = psC.tile([128, 512], F32, tag="bc", name="ps_bc")
                nc.tensor.matmul(ps_bc[:, :], cB('wones'),
                                 sigrow[:, bass.ts(t, 512)],
                                 start=True, stop=True)
                if t % 2 == 0:
                    nc.scalar.activation(out=sigb[:, 0, bass.ts(t, 512)],
                                         in_=ps_bc[:, :], func=AF.Copy)
                else:
                    nc.vector.tensor_copy(out=sigb[:, 0, bass.ts(t, 512)],
                                          in_=ps_bc[:, :])

            # final: out = zs * sigb + xp (Pool takes group 0's multiply)
            for g in range(4):
                vt = sp.tile([128, 2, 1024], BF16, tag="vt", name="vt")
                sl = bass.ts(g, 1024)
                sgb = sigb[:, :, sl].broadcast_to([128, 2, 1024])
                eng = nc.gpsimd if g == 0 else nc.vector
                eng.tensor_tensor(out=vt[:, :, :], in0=z_t[:, :, sl], in1=sgb,
                                  op=ALU.mult)
                nc.vector.tensor_tensor(out=vt[:, :, :], in0=vt[:, :, :],
                                        in1=xp_t[:, :, sl], op=ALU.add)
                nc.sync.dma_start(
                    out=out_d[:, sl].rearrange("(two p) n -> p two n", two=2),
                    in_=vt[:, :, :])
        stack.close()
        for fr in reversed(_frees):
            fr()
    nc.compile()
    return nc


_CACHE = {}


def kernel(**inputs):
    inp = {k: np.asarray(v) for k, v in inputs.items()}
    f = fold_params(inp)
    key = round(f['w_fuse'], 9)
    if key not in _CACHE:
        _CACHE[key] = build_nc(f['w_fuse'])
    nc = _CACHE[key]

    B = inp['x'].shape[0]
    in_maps = []
    for b in range(B):
        xb = inp['x'][b].reshape(256, N).astype(np.float32)
        m = {
            'x': np.ascontiguousarray(
                xb.reshape(2, 128, N).transpose(1, 0, 2)).astype(ml_dtypes.bfloat16),
            'x0': np.ascontiguousarray(
                inp['x0'][b].reshape(128, N)).astype(ml_dtypes.bfloat16),
            'cbA': f['cbA'], 'cbB': f['cbB'], 'cf': f['cf'],
        }
        in_maps.append(m)

    res = run_bass_kernel_spmd(nc, in_maps, core_ids=list(range(B)))
    out = np.stack([np.asarray(res.results[b]['out'], dtype=np.float32
                               ).reshape(256, H, W) for b in range(B)])
    return out


# revision 22
# speedup vs baseline: 1.4832x; 1.0285x over previous
"""Trainium2 Bass kernel for nn_MDFO (CNL + PNL non-local blocks + CBAM + fusion).

Restructured v3 (pure data-parallel, B=8 over 8 cores, params replicated):
  - bf16 inputs (x, x0) uploaded from host; bf16 output, fp32 on host.
  - (1-w)*x residual computed on the otherwise-idle Pool engine.
  - all constants packed into three blob DMAs (early-bf16, late-bf16, f32).
  - y and g_x never materialized: runtime weight folds WA/WC/w_ta/WD with
    rank-1 bias fixups; T2/Y2/S2 stacked layouts halve matmul+copy counts.
  - CBAM mean via matmul accum_out, max via rolling bf16 max accumulator.
  - mean map from raw z with ca as the matmul stationary (starts at ca).
  - w_fuse folded into the sig-broadcast stationary vector.
  - final out = zs*sigb + xp with wide bf16 tensor_tensor ops.
"""
import sys

import numpy as np

sys.path.insert(0, "/opt/trn_rl_repo")

import ml_dtypes  # noqa: E402

import concourse.bass as bass  # noqa: E402
import concourse.bacc as bacc  # noqa: E402
import concourse.tile as tile  # noqa: E402
from concourse import mybir  # noqa: E402
from concourse.bass_utils import run_bass_kernel_spmd  # noqa: E402

EPS = 1e-5
F32 = mybir.dt.float32
F32R = mybir.dt.float32r
BF16 = mybir.dt.bfloat16
AF = mybir.ActivationFunctionType
ALU = mybir.AluOpType

Ch, Cl, H, W = 256, 128, 64, 64
N = H * W            # 4096
M = N // 2           # 2048
r = Cl // 2          # 64

# blob layouts: name -> (col offset, cols, rows)
CBA_COLS = 1152  # early bf16 blob
CBA = {'w_x0cat': (0, 256, 128), 'b_x0cat': (256, 256, 128),
       'w_th_bf': (512, 256, 128), 'b_th_row': (768, 128, 1),
       'ones1': (896, 128, 1), 'ident_bf': (1024, 128, 128)}
CBB_COLS = 1856  # late bf16 blob
CBB = {'w_th2': (0, 128, 128), 'w_pnlW': (128, 256, 128),
       'Kcat2': (384, 896, 64), 'Sdy7': (1280, 448, 64),
       'wones': (1728, 128, 1)}
CF_COLS = 869    # f32 blob
CF = {'ident': (0, 128, 128), 'w_gT': (128, 128, 128), 'b_g': (256, 2, 128),
      'w_cnlW': (258, 256, 128), 'w_tyT': (514, 64, 128),
      'b_th2': (578, 1, 64), 'b2': (579, 2, 128), 'fc1T': (581, 32, 128),
      'fc2T': (613, 256, 16)}


def _R(ap):
    return ap.bitcast(F32R)


def fold_params(inp):
    """Host-side constant folding into three blob arrays."""
    f = {}
    scale1 = inp['cnl_bn_g'] / np.sqrt(inp['cnl_bn_v'] + EPS)
    cnl_bf = (inp['cnl_W_b'] * scale1 + inp['cnl_bn_b']
              - inp['cnl_bn_m'] * scale1).astype(np.float32)
    scale2 = inp['pnl_bn_g'] / np.sqrt(inp['pnl_bn_v'] + EPS)
    pnl_bf = (inp['pnl_W_b'] * scale2 + inp['pnl_bn_b']
              - inp['pnl_bn_m'] * scale2).astype(np.float32)
    w_fuse = float(inp['fusion_weight'])
    f['w_fuse'] = w_fuse

    cbA = np.zeros((128, CBA_COLS), dtype=np.float32)
    cbB = np.zeros((128, CBB_COLS), dtype=np.float32)
    cf = np.zeros((128, CF_COLS), dtype=np.float32)

    def put(blob, table, name, arr):
        off, cols, rows = table[name]
        blob[:rows, off:off + cols] = arr

    put(cbA, CBA, 'w_x0cat', np.concatenate([
        inp['cnl_phi_w'].T, inp['pnl_phi_w'].T, (inp['pnl_g_w'] / M).T],
        axis=1))
    brow = np.concatenate([inp['cnl_phi_b'], inp['pnl_phi_b'],
                           inp['pnl_g_b'] / M])
    put(cbA, CBA, 'b_x0cat', np.tile(brow[None, :], (128, 1)))
    thT = inp['cnl_theta_w'].T
    put(cbA, CBA, 'w_th_bf', np.concatenate([thT[:128], thT[128:]], axis=1))
    put(cbA, CBA, 'b_th_row', inp['cnl_theta_b'][None, :])
    put(cbA, CBA, 'ones1', np.ones((1, 128), dtype=np.float32))
    put(cbA, CBA, 'ident_bf', np.eye(128, dtype=np.float32))

    th2 = inp['pnl_theta_w'].T
    put(cbB, CBB, 'w_th2', np.concatenate([th2[:128], th2[128:]], axis=1))
    w_pnlW = (scale2[:, None] * inp['pnl_W_w']).T
    put(cbB, CBB, 'w_pnlW', np.concatenate([w_pnlW, w_pnlW], axis=0))
    # sa conv banded mats; only 1/256 fold on the mean channel (no w folds)
    sa_w = np.asarray(inp['sa_conv_w'][0], dtype=np.float32).copy()
    sa_w[0] /= 256.0
    Kcat = np.zeros((2, 64, 7 * 64), dtype=np.float32)
    for ch in range(2):
        for dy in range(7):
            for dx in range(7):
                w_ = sa_w[ch, dy, dx]
                if w_ == 0.0:
                    continue
                for x in range(64):
                    xq = x + dx - 3
                    if 0 <= xq < 64:
                        Kcat[ch, xq, dy * 64 + x] = w_
    put(cbB, CBB, 'Kcat2', np.concatenate([Kcat[0], Kcat[1]], axis=1))
    Sdy = np.zeros((64, 7 * 64), dtype=np.float32)
    for dy in range(7):
        for y in range(64):
            yp = y + dy - 3
            if 0 <= yp < 64:
                Sdy[yp, dy * 64 + y] = 1.0
    perm = np.array([2 * (q % 32) + q // 32 for q in range(64)])
    put(cbB, CBB, 'Sdy7', Sdy[perm, :])
    put(cbB, CBB, 'wones', np.full((1, 128), w_fuse, dtype=np.float32))

    put(cf, CF, 'ident', np.eye(128, dtype=np.float32))
    put(cf, CF, 'w_gT', inp['cnl_g_w'] / Cl)
    bgc = (inp['cnl_g_b'] / Cl)[:, None]
    put(cf, CF, 'b_g', np.concatenate([bgc, bgc], axis=1))
    put(cf, CF, 'w_cnlW', (scale1[:, None] * inp['cnl_W_w']).T)
    put(cf, CF, 'w_tyT', (inp['pnl_theta_w'] @ (scale1[:, None] * inp['cnl_W_w'])).T)
    put(cf, CF, 'b_th2', (inp['pnl_theta_b'] + inp['pnl_theta_w'] @ cnl_bf)[:, None])
    bias2 = (pnl_bf + cnl_bf)
    put(cf, CF, 'b2', np.stack([bias2[:128], bias2[128:]], axis=1))
    fc1 = inp['ca_fc1_w'].T
    put(cf, CF, 'fc1T', np.concatenate([fc1[:128], fc1[128:]], axis=1))
    put(cf, CF, 'fc2T', inp['ca_fc2_w'].T)

    f['cbA'] = cbA.astype(ml_dtypes.bfloat16)
    f['cbB'] = cbB.astype(ml_dtypes.bfloat16)
    f['cf'] = cf.astype(np.float32)
    return f


def build_nc(w_fuse):
    nc = bacc.Bacc(None)
    x_d = nc.declare_dram_parameter("x", [128, 2, N], BF16, isOutput=False)
    x0_d = nc.declare_dram_parameter("x0", [128, N], BF16, isOutput=False)
    cbA_d = nc.declare_dram_parameter("cbA", [128, CBA_COLS], BF16, isOutput=False)
    cbB_d = nc.declare_dram_parameter("cbB", [128, CBB_COLS], BF16, isOutput=False)
    cf_d = nc.declare_dram_parameter("cf", [128, CF_COLS], F32R, isOutput=False)
    out_d = nc.declare_dram_parameter("out", [256, N], BF16, isOutput=True)
    smean_d = nc.dram_tensor("smean", [1, N], BF16)
    ssig_d = nc.dram_tensor("ssig", [1, N], BF16)

    with tile.TileContext(nc) as tc:
        _frees = []

        def _keep(pair):
            _frees.append(pair[1])
            return pair[0]

        # ---- persistent SBUF tensors ----
        x_t = _keep(tc.tile([128, 2, N], BF16, name="x_t"))
        xp_t = _keep(tc.tile([128, 2, N], BF16, name="xp_t"))
        x0_t = _keep(tc.tile([128, N], BF16, name="x0_t"))
        cbA_t = _keep(tc.tile([128, CBA_COLS], BF16, name="cbA_t"))
        cbB_t = _keep(tc.tile([128, CBB_COLS], BF16, name="cbB_t"))
        cf_t = _keep(tc.tile([128, CF_COLS], F32R, name="cf_t"))
        x0cat = _keep(tc.tile([128, 32, 256], BF16, name="x0cat"))
        thT = _keep(tc.tile([128, 32, 128], BF16, name="thT"))
        attT = _keep(tc.tile([128, 128], F32R, name="attT"))
        att_s = _keep(tc.tile([128, 128], F32R, name="att_s"))
        WA_s = _keep(tc.tile([128, 256], F32R, name="WA_s"))
        WC_s = _keep(tc.tile([128, 256], BF16, name="WC_s"))
        wta_s = _keep(tc.tile([128, 64], F32R, name="wta_s"))
        WD_s = _keep(tc.tile([128, 64], BF16, name="WD_s"))
        S2_s = _keep(tc.tile([128, 128], BF16, name="S2_s"))
        T2 = _keep(tc.tile([128, M], BF16, name="T2"))
        Y2 = _keep(tc.tile([128, M], BF16, name="Y2"))
        z_t = _keep(tc.tile([128, 2, N], BF16, name="z_t"))
        bz = _keep(tc.tile([128, 2], F32, name="bz"))
        bT2 = _keep(tc.tile([128, 1], F32, name="bT2"))
        psum_cols = _keep(tc.tile([128, 2, 4], F32, name="psum_cols"))
        macc = _keep(tc.tile([128, 2, 512], BF16, name="macc"))
        V_t = _keep(tc.tile([128, 2, 2], F32, name="V_t"))
        h_t = _keep(tc.tile([16, 2], F32, name="h_t"))
        ca_t = _keep(tc.tile([128, 2], F32, name="ca_t"))
        ca_bf = _keep(tc.tile([128, 2], BF16, name="ca_bf"))
        tmp1 = _keep(tc.tile([128, 4], F32, name="tmp1"))
        tA = _keep(tc.tile([128, N], BF16, name="tA"))
        PM = _keep(tc.tile([128, 32], BF16, name="PM"))
        PModd = _keep(tc.tile([64, 32], BF16, name="PModd"))
        m2d_sb = _keep(tc.tile([64, 64], BF16, name="m2d_sb"))
        meanrow = _keep(tc.tile([1, N], BF16, name="meanrow"))
        sigrow = _keep(tc.tile([1, N], BF16, name="sigrow"))
        mapT_mean = _keep(tc.tile([64, 64], BF16, name="mapT_mean"))
        mapT_meanP = _keep(tc.tile([64, 64], BF16, name="mapT_meanP"))
        R_sb = _keep(tc.tile([64, 448], BF16, name="R_sb"))
        sig2d = _keep(tc.tile([64, 64], BF16, name="sig2d"))
        sigb = _keep(tc.tile([128, 1, N], BF16, name="sigb"))

        def cA(name, rows=None):
            off, cols, rws = CBA[name]
            return cbA_t[0:(rows or rws), off:off + cols]

        def cB(name, rows=None):
            off, cols, rws = CBB[name]
            return cbB_t[0:(rows or rws), off:off + cols]

        def cF(name, rows=None):
            off, cols, rws = CF[name]
            return cf_t[0:(rows or rws), off:off + cols]

        from contextlib import ExitStack
        stack = ExitStack()

        # ---- DMAs: first pixel group + early consts, then the rest ----
        nc.sync.dma_start(out=x0_t[:, 0:512], in_=x0_d[:, 0:512])
        nc.sync.dma_start(out=x_t[:, :, 0:512], in_=x_d[:, :, 0:512])
        nc.sync.dma_start(out=cbA_t[:, :], in_=cbA_d[:, :])
        nc.sync.dma_start(out=x0_t[:, 512:2048], in_=x0_d[:, 512:2048])
        nc.sync.dma_start(out=x_t[:, :, 512:2048], in_=x_d[:, :, 512:2048])
        nc.sync.dma_start(out=x0_t[:, 2048:4096], in_=x0_d[:, 2048:4096])
        nc.sync.dma_start(out=x_t[:, :, 2048:4096], in_=x_d[:, :, 2048:4096])
        nc.sync.dma_start(out=cbB_t[:, :], in_=cbB_d[:, :])
        nc.sync.dma_start(out=cf_t[:, :], in_=cf_d[:, :])

        sp = stack.enter_context(tc.tile_pool(name="sp", bufs=3))

        # warm the sigmoid act-table set (contains identity/copy/relu too)
        warm = sp.tile([1, 8], F32, tag="warm", name="warm", bufs=1)
        nc.vector.memset(warm[:, :], 0.0)
        nc.scalar.activation(out=warm[:, :], in_=warm[:, :], func=AF.Sigmoid)

        # =========== Stage A: x0cat + thT + att; then S blocks ===========
        with tc.tile_pool(name="psA", bufs=2, space="PSUM") as psA, \
             tc.tile_pool(name="ps1", bufs=1, space="PSUM") as ps1:
            att_ps = ps1.tile([128, 128], F32, tag="att", name="att_ps")
            ps_s = ps1.tile([64, 256], F32, tag="S2", name="ps_s")
            for t8 in range(8):
                ps_x0c = psA.tile([128, 1024], F32, tag="x0c", name="ps_x0c")
                ps_tht = psA.tile([128, 512], F32, tag="tht", name="ps_tht")
                for sub in range(4):
                    i = 4 * t8 + sub
                    nc.tensor.matmul(ps_x0c[:, bass.ts(sub, 256)],
                                     x0_t[:, bass.ts(i, 128)], cA('w_x0cat'),
                                     start=True, stop=True)
                    nc.tensor.matmul(ps_tht[:, bass.ts(sub, 128)],
                                     cA('ones1'), cA('b_th_row'),
                                     start=True, stop=False)
                    nc.tensor.matmul(ps_tht[:, bass.ts(sub, 128)],
                                     x_t[:, 0, bass.ts(i, 128)],
                                     cA('w_th_bf')[:, 0:128],
                                     start=False, stop=False)
                    nc.tensor.matmul(ps_tht[:, bass.ts(sub, 128)],
                                     x_t[:, 1, bass.ts(i, 128)],
                                     cA('w_th_bf')[:, 128:256],
                                     start=False, stop=True)
                nc.vector.tensor_tensor(
                    out=x0cat[:, 4 * t8:4 * t8 + 4, :],
                    in0=ps_x0c[:, :].rearrange("p (a c) -> p a c", c=256),
                    in1=cA('b_x0cat').rearrange("p (a c) -> p a c", c=256
                                                ).broadcast_to([128, 4, 256]),
                    op=ALU.add)
                nc.scalar.activation(
                    out=thT[:, 4 * t8:4 * t8 + 4, :],
                    in_=ps_tht[:, :].rearrange("p (a c) -> p a c", c=128),
                    func=AF.Copy)
                for sub in range(4):
                    i = 4 * t8 + sub
                    nc.tensor.matmul(att_ps[:, :], x0cat[:, i, 0:128],
                                     thT[:, i, :], start=(i == 0), stop=(i == 31))
            # S blocks: consecutive emission (interleaving the four shared-bank
            # psum streams with other matmuls corrupts the accumulation)
            for j in range(16):
                st = (j == 0)
                sp_ = (j == 15)
                PTa = x0cat[:, j, 128:192]
                PTb = x0cat[:, j + 16, 128:192]
                GTa = x0cat[:, j, 192:256]
                GTb = x0cat[:, j + 16, 192:256]
                nc.tensor.matmul(ps_s[:, 0:64], PTa, GTa, start=st, stop=sp_)
                nc.tensor.matmul(ps_s[:, 64:128], PTa, GTb, start=st, stop=sp_)
                nc.tensor.matmul(ps_s[:, 128:192], PTb, GTa, start=st, stop=sp_)
                nc.tensor.matmul(ps_s[:, 192:256], PTb, GTb, start=st, stop=sp_)
            nc.scalar.copy(out=attT[:, :], in_=att_ps[:, :])
            nc.vector.tensor_copy(out=S2_s[0:64, :], in_=ps_s[:, 0:128])
            nc.vector.tensor_copy(out=S2_s[64:128, :], in_=ps_s[:, 128:256])

        # xp = (1-w) * x on the idle Pool engine
        for g in range(4):
            nc.gpsimd.tensor_scalar(out=xp_t[:, :, bass.ts(g, 1024)],
                                    in0=x_t[:, :, bass.ts(g, 1024)],
                                    scalar1=1.0 - w_fuse, scalar2=None,
                                    op0=ALU.mult)

        # =========== folds + T + Y + z + channel attention ===========
        with tc.tile_pool(name="psB", bufs=2, space="PSUM") as psB:
            ps_at = psB.tile([128, 128], F32R, tag="sm", name="ps_at")
            nc.tensor.transpose(_R(ps_at[:, :]), attT[:, :], _R(cF('ident')))
            nc.scalar.copy(out=att_s[:, :], in_=ps_at[:, :])
            ps_wt = psB.tile([128, 64], F32, tag="sm", name="ps_wt")
            nc.tensor.matmul(ps_wt[:, :], att_s[:, :], _R(cF('w_tyT')),
                             start=True, stop=True)
            nc.scalar.copy(out=wta_s[:, :], in_=ps_wt[:, :])
            ps_wd = psB.tile([128, 64], F32, tag="sm", name="ps_wd")
            nc.tensor.matmul(ps_wd[:, :], _R(cF('w_gT')), wta_s[:, :],
                             start=True, stop=True)
            nc.vector.tensor_copy(out=WD_s[:, :], in_=ps_wd[:, :])
            ps_bt = psB.tile([64, 2], F32, tag="sm", name="ps_bt")
            nc.tensor.matmul(ps_bt[:, :], wta_s[:, :], _R(cF('b_g')),
                             start=True, stop=True)
            nc.vector.tensor_tensor(out=bT2[0:64, :], in0=ps_bt[:, 0:1],
                                    in1=cF('b_th2').bitcast(F32), op=ALU.add)
            nc.vector.tensor_copy(out=bT2[64:128, :], in_=bT2[0:64, :])
            ps_wa = psB.tile([128, 256], F32, tag="sm", name="ps_wa")
            nc.tensor.matmul(ps_wa[:, :], att_s[:, :], _R(cF('w_cnlW')),
                             start=True, stop=True)
            nc.scalar.copy(out=WA_s[:, :], in_=ps_wa[:, :])
            ps_wc = psB.tile([128, 256], F32, tag="sm", name="ps_wc")
            nc.tensor.matmul(ps_wc[:, :], _R(cF('w_gT')), WA_s[:, :],
                             start=True, stop=True)
            nc.vector.tensor_copy(out=WC_s[:, :], in_=ps_wc[:, :])
            ps_bb = psB.tile([128, 4], F32, tag="sm", name="ps_bb")
            nc.tensor.matmul(ps_bb[:, 0:2], WA_s[:, 0:128], _R(cF('b_g')),
                             start=True, stop=True)
            nc.tensor.matmul(ps_bb[:, 2:4], WA_s[:, 128:256], _R(cF('b_g')),
                             start=True, stop=True)
            nc.vector.tensor_tensor(out=bz[:, 0:1], in0=ps_bb[:, 0:1],
                                    in1=cF('b2')[:, 0:1].bitcast(F32), op=ALU.add)
            nc.vector.tensor_tensor(out=bz[:, 1:2], in0=ps_bb[:, 2:3],
                                    in1=cF('b2')[:, 1:2].bitcast(F32), op=ALU.add)

            # ---- T2 [128, M] ----
            for tm in range(4):
                ps_T = psB.tile([128, 512], F32, tag="TY", name="ps_T")
                for h in range(2):
                    base = h * M + tm * 512
                    o = ps_T[64 * h:64 * h + 64, :]
                    nc.tensor.matmul(o, WD_s[:, :], x0_t[:, base:base + 512],
                                     start=True, stop=False)
                    nc.tensor.matmul(o, cB('w_th2')[:, 0:64],
                                     x_t[:, 0, base:base + 512],
                                     start=False, stop=False)
                    nc.tensor.matmul(o, cB('w_th2')[:, 64:128],
                                     x_t[:, 1, base:base + 512],
                                     start=False, stop=True)
                nc.scalar.activation(out=T2[:, bass.ts(tm, 512)], in_=ps_T[:, :],
                                     func=AF.Identity, bias=bT2[:, :])

            # ---- Y2 [128, M] ----
            for tm in range(4):
                ps_Y = psB.tile([128, 512], F32, tag="TY", name="ps_Y")
                nc.tensor.matmul(ps_Y[:, :], S2_s[:, :], T2[:, bass.ts(tm, 512)],
                                 start=True, stop=True)
                nc.scalar.activation(out=Y2[:, bass.ts(tm, 512)], in_=ps_Y[:, :],
                                     func=AF.Copy)

            # ---- z [128, 2, N] bf16, paired tiles per psum ----
            for t2 in range(4):
                for ch in range(2):
                    ps_z = psB.tile([128, 1024], F32, tag="z", name="ps_z")
                    for k in range(2):
                        t = 2 * t2 + k
                        h = t // 4
                        mbase = (t % 4) * 512
                        o = ps_z[:, bass.ts(k, 512)]
                        nc.tensor.matmul(o, cB('w_pnlW')[64 * h:64 * h + 64,
                                                         bass.ts(ch, 128)],
                                         Y2[64 * h:64 * h + 64, mbase:mbase + 512],
                                         start=True, stop=False)
                        nc.tensor.matmul(o, WC_s[:, bass.ts(ch, 128)],
                                         x0_t[:, bass.ts(t, 512)],
                                         start=False, stop=(ch == 1))
                        if ch == 0:
                            nc.tensor.matmul(o, cA('ident_bf'),
                                             x_t[:, 0, bass.ts(t, 512)],
                                             start=False, stop=True)
                    if ch == 0:
                        nc.scalar.activation(
                            out=z_t[:, 0, bass.ts(t2, 1024)], in_=ps_z[:, :],
                            func=AF.Identity, bias=bz[:, 0:1],
                            accum_out=psum_cols[:, 0, t2:t2 + 1])
                    else:
                        nc.vector.scalar_tensor_tensor(
                            out=z_t[:, 1, bass.ts(t2, 1024)], in0=ps_z[:, :],
                            scalar=bz[:, 1:2],
                            in1=x_t[:, 1, bass.ts(t2, 1024)], op0=ALU.add,
                            op1=ALU.add, accum_out=psum_cols[:, 1, t2:t2 + 1])
                # rolling channel-wise max accumulator (two 512-steps)
                for k in range(2):
                    t = 2 * t2 + k
                    if t == 0:
                        nc.vector.tensor_copy(out=macc[:, :, :],
                                              in_=z_t[:, :, 0:512])
                    else:
                        nc.vector.tensor_tensor(
                            out=macc[:, :, :], in0=macc[:, :, :],
                            in1=z_t[:, :, bass.ts(t, 512)], op=ALU.max)

            # ---- CBAM channel attention (compressed chain) ----
            nc.vector.reduce_max(out=tmp1[:, 0:2], in_=macc[:, :, :],
                                 axis=mybir.AxisListType.X)
            nc.scalar.activation(out=V_t[:, :, 1:2], in_=tmp1[:, 0:2],
                                 func=AF.Identity)
            nc.vector.reduce_sum(out=tmp1[:, 2:4], in_=psum_cols[:, :, :],
                                 axis=mybir.AxisListType.X)
            nc.scalar.activation(out=V_t[:, :, 0:1], in_=tmp1[:, 2:4],
                                 func=AF.Identity, scale=1.0 / float(N))
            ps_f1 = psB.tile([16, 2], F32, tag="sm", name="ps_f1")
            nc.tensor.matmul(ps_f1[:, :], cF('fc1T')[:, 0:16].bitcast(F32), V_t[:, 0, :],
                             start=True, stop=False)
            nc.tensor.matmul(ps_f1[:, :], cF('fc1T')[:, 16:32].bitcast(F32), V_t[:, 1, :],
                             start=False, stop=True)
            nc.scalar.activation(out=h_t[:, :], in_=ps_f1[:, :], func=AF.Relu)
            for ch in range(2):
                ps_f2 = psB.tile([128, 2], F32, tag="sm", name="ps_f2")
                nc.tensor.matmul(ps_f2[:, :], cF('fc2T')[:, bass.ts(ch, 128)].bitcast(F32),
                                 h_t[:, :], start=True, stop=True)
                nc.vector.reduce_sum(out=tmp1[:, ch:ch + 1], in_=ps_f2[:, :],
                                     axis=mybir.AxisListType.X)
            nc.scalar.activation(out=ca_t[:, :], in_=tmp1[:, 0:2],
                                 func=AF.Sigmoid)
            nc.vector.tensor_copy(out=ca_bf[:, :], in_=ca_t[:, :])

        # =========== maps + sa conv + final ===========
        with tc.tile_pool(name="psC", bufs=2, space="PSUM") as psC:
            # zs = z * ca in place: Act ch0; ch1 split Pool/DVE
            for g in range(2):
                nc.scalar.activation(out=z_t[:, 0, bass.ts(g, 2048)],
                                     in_=z_t[:, 0, bass.ts(g, 2048)],
                                     func=AF.Copy, scale=ca_t[:, 0:1])
            nc.gpsimd.tensor_scalar(out=z_t[:, 1, 0:2048],
                                    in0=z_t[:, 1, 0:2048],
                                    scalar1=ca_t[:, 1:2], scalar2=None,
                                    op0=ALU.mult)
            nc.vector.tensor_scalar(out=z_t[:, 1, 2048:4096],
                                    in0=z_t[:, 1, 2048:4096],
                                    scalar1=ca_t[:, 1:2], scalar2=None,
                                    op0=ALU.mult)
            # mean map from raw z with ca as stationary (runs right at ca)
            for t in range(8):
                ps_m = psC.tile([1, 512], F32, tag="sm2", name="ps_m")
                nc.tensor.matmul(ps_m[:, :], ca_bf[:, 0:1],
                                 z_t[:, 0, bass.ts(t, 512)],
                                 start=True, stop=False)
                nc.tensor.matmul(ps_m[:, :], ca_bf[:, 1:2],
                                 z_t[:, 1, bass.ts(t, 512)],
                                 start=False, stop=True)
                if t % 2 == 0:
                    nc.vector.tensor_copy(out=meanrow[:, bass.ts(t, 512)],
                                          in_=ps_m[:, :])
                else:
                    nc.scalar.activation(out=meanrow[:, bass.ts(t, 512)],
                                         in_=ps_m[:, :], func=AF.Copy)
            nc.scalar.dma_start(out=smean_d[:, :], in_=meanrow[:, :])
            nc.scalar.dma_start(
                out=m2d_sb[:, :],
                in_=smean_d[:, :].rearrange("p (a b) -> (p a) b", b=64))
            ps_tm = psC.tile([64, 64], BF16, tag="sm2", name="ps_tm")
            nc.tensor.transpose(ps_tm[:, :], m2d_sb[:, :],
                                cA('ident_bf')[0:64, 0:64])
            nc.scalar.activation(out=mapT_mean[:, :], in_=ps_tm[:, :], func=AF.Copy)
            nc.scalar.activation(
                out=mapT_meanP[:, :],
                in_=mapT_mean[:, :].rearrange("p (c two) -> p two c", two=2),
                func=AF.Copy)

            # tA = max over channel chunks
            for g in range(2):
                nc.vector.tensor_tensor(out=tA[:, bass.ts(g, 2048)],
                                        in0=z_t[:, 0, bass.ts(g, 2048)],
                                        in1=z_t[:, 1, bass.ts(g, 2048)],
                                        op=ALU.max)

            # max map: transposes + per-group reduce
            for b4 in range(8):
                ps_tx = psC.tile([128, 4, 128], BF16, tag="tx", name="ps_tx")
                for k in range(4):
                    gidx = 4 * b4 + k
                    nc.tensor.transpose(ps_tx[:, k, :], tA[:, bass.ts(gidx, 128)],
                                        cA('ident_bf'))
                nc.vector.reduce_max(out=PM[:, bass.ts(b4, 4)],
                                     in_=ps_tx[:, :, :],
                                     axis=mybir.AxisListType.X)
            nc.sync.dma_start(out=PModd[:, :], in_=PM[64:128, :])

            # sa conv (banded) + sigmoid
            ps_R = psC.tile([64, 448], F32, tag="sm2", name="ps_R")
            nc.tensor.matmul(ps_R[:, :], mapT_meanP[:, :], cB('Kcat2')[:, 0:448],
                             start=True, stop=False)
            nc.tensor.matmul(ps_R[0:32, :], PM[0:64, :], cB('Kcat2')[:, 448:896],
                             start=False, stop=False)
            nc.tensor.matmul(ps_R[32:64, :], PModd[:, :], cB('Kcat2')[:, 448:896],
                             start=False, stop=True, tile_position=(0, 32))
            nc.scalar.activation(out=R_sb[:, :], in_=ps_R[:, :], func=AF.Copy)
            ps_sa = psC.tile([64, 64], F32, tag="sm2", name="ps_sa")
            for dy in range(7):
                nc.tensor.matmul(ps_sa[:, :], cB('Sdy7')[:, bass.ts(dy, 64)],
                                 R_sb[:, bass.ts(dy, 64)],
                                 start=(dy == 0), stop=(dy == 6))
            nc.scalar.activation(out=sig2d[:, :], in_=ps_sa[:, :], func=AF.Sigmoid)
            for hh in range(2):
                nc.sync.dma_start(
                    out=ssig_d[:, bass.ts(hh, 2048)].rearrange(
                        "p (a b) -> (p a) b", b=64),
                    in_=sig2d[32 * hh:32 * hh + 32, :])
                nc.sync.dma_start(out=sigrow[:, bass.ts(hh, 2048)],
                                  in_=ssig_d[:, bass.ts(hh, 2048)])

            # sig broadcast (w_fuse folded into the stationary ones)
            for t in range(8):
                ps_bc = psC.tile([128, 512], F32, tag="bc", name="ps_bc")
                nc.tensor.matmul(ps_bc[:, :], cB('wones'),
                                 sigrow[:, bass.ts(t, 512)],
                                 start=True, stop=True)
                if t % 2 == 0:
                    nc.scalar.activation(out=sigb[:, 0, bass.ts(t, 512)],
                                         in_=ps_bc[:, :], func=AF.Copy)
                else:
                    nc.vector.tensor_copy(out=sigb[:, 0, bass.ts(t, 512)],
                                          in_=ps_bc[:, :])

            # final: out = zs * sigb + xp (Pool takes group 0's multiply)
            for g in range(4):
                vt = sp.tile([128, 2, 1024], BF16, tag="vt", name="vt")
                sl = bass.ts(g, 1024)
                sgb = sigb[:, :, sl].broadcast_to([128, 2, 1024])
                eng = nc.gpsimd if g == 0 else nc.vector
                eng.tensor_tensor(out=vt[:, :, :], in0=z_t[:, :, sl], in1=sgb,
                                  op=ALU.mult)
                nc.vector.tensor_tensor(out=vt[:, :, :], in0=vt[:, :, :],
                                        in1=xp_t[:, :, sl], op=ALU.add)
                nc.sync.dma_start(
                    out=out_d[:, sl].rearrange("(two p) n -> p two n", two=2),
                    in_=vt[:, :, :])
        stack.close()
        for fr in reversed(_frees):
            fr()
    nc.compile()
    return nc


_CACHE = {}


def kernel(**inputs):
    inp = {k: np.asarray(v) for k, v in inputs.items()}
    f = fold_params(inp)
    key = round(f['w_fuse'], 9)
    if key not in _CACHE:
        _CACHE[key] = build_nc(f['w_fuse'])
    nc = _CACHE[key]

    B = inp['x'].shape[0]
    in_maps = []
    for b in range(B):
        xb = inp['x'][b].reshape(256, N).astype(np.float32)
        m = {
            'x': np.ascontiguousarray(
                xb.reshape(2, 128, N).transpose(1, 0, 2)).astype(ml_dtypes.bfloat16),
            'x0': np.ascontiguousarray(
                inp['x0'][b].reshape(128, N)).astype(ml_dtypes.bfloat16),
            'cbA': f['cbA'], 'cbB': f['cbB'], 'cf': f['cf'],
        }
        in_maps.append(m)

    res = run_bass_kernel_spmd(nc, in_maps, core_ids=list(range(B)))
    out = np.stack([np.asarray(res.results[b]['out'], dtype=np.float32
                               ).reshape(256, H, W) for b in range(B)])
    return out


# revision 25
# speedup vs baseline: 1.4840x; 1.0005x over previous
"""Trainium2 Bass kernel for nn_MDFO (CNL + PNL non-local blocks + CBAM + fusion).

Restructured v3 (pure data-parallel, B=8 over 8 cores, params replicated):
  - bf16 inputs (x, x0) uploaded from host; bf16 output, fp32 on host.
  - (1-w)*x residual computed on the otherwise-idle Pool engine.
  - all constants packed into three blob DMAs (early-bf16, late-bf16, f32).
  - y and g_x never materialized: runtime weight folds WA/WC/w_ta/WD with
    rank-1 bias fixups; T2/Y2/S2 stacked layouts halve matmul+copy counts.
  - CBAM mean via matmul accum_out, max via rolling bf16 max accumulator.
  - mean map from raw z with ca as the matmul stationary (starts at ca).
  - w_fuse folded into the sig-broadcast stationary vector.
  - final out = zs*sigb + xp with wide bf16 tensor_tensor ops.
"""
import sys

import numpy as np

sys.path.insert(0, "/opt/trn_rl_repo")

import ml_dtypes  # noqa: E402

import concourse.bass as bass  # noqa: E402
import concourse.bacc as bacc  # noqa: E402
import concourse.tile as tile  # noqa: E402
from concourse import mybir  # noqa: E402
from concourse.bass_utils import run_bass_kernel_spmd  # noqa: E402

EPS = 1e-5
F32 = mybir.dt.float32
F32R = mybir.dt.float32r
BF16 = mybir.dt.bfloat16
AF = mybir.ActivationFunctionType
ALU = mybir.AluOpType

Ch, Cl, H, W = 256, 128, 64, 64
N = H * W            # 4096
M = N // 2           # 2048
r = Cl // 2          # 64

# blob layouts: name -> (col offset, cols, rows)
CBA_COLS = 1152  # early bf16 blob
CBA = {'w_x0cat': (0, 256, 128), 'b_x0cat': (256, 256, 128),
       'w_th_bf': (512, 256, 128), 'b_th_row': (768, 128, 1),
       'ones1': (896, 128, 1), 'ident_bf': (1024, 128, 128)}
CBB_COLS = 1856  # late bf16 blob
CBB = {'w_th2': (0, 128, 128), 'w_pnlW': (128, 256, 128),
       'Kcat2': (384, 896, 64), 'Sdy7': (1280, 448, 64),
       'wones': (1728, 128, 1)}
CF_COLS = 869    # f32 blob
CF = {'ident': (0, 128, 128), 'w_gT': (128, 128, 128), 'b_g': (256, 2, 128),
      'w_cnlW': (258, 256, 128), 'w_tyT': (514, 64, 128),
      'b_th2': (578, 1, 64), 'b2': (579, 2, 128), 'fc1T': (581, 32, 128),
      'fc2T': (613, 256, 16)}


def _R(ap):
    return ap.bitcast(F32R)


def fold_params(inp):
    """Host-side constant folding into three blob arrays."""
    f = {}
    scale1 = inp['cnl_bn_g'] / np.sqrt(inp['cnl_bn_v'] + EPS)
    cnl_bf = (inp['cnl_W_b'] * scale1 + inp['cnl_bn_b']
              - inp['cnl_bn_m'] * scale1).astype(np.float32)
    scale2 = inp['pnl_bn_g'] / np.sqrt(inp['pnl_bn_v'] + EPS)
    pnl_bf = (inp['pnl_W_b'] * scale2 + inp['pnl_bn_b']
              - inp['pnl_bn_m'] * scale2).astype(np.float32)
    w_fuse = float(inp['fusion_weight'])
    f['w_fuse'] = w_fuse

    cbA = np.zeros((128, CBA_COLS), dtype=np.float32)
    cbB = np.zeros((128, CBB_COLS), dtype=np.float32)
    cf = np.zeros((128, CF_COLS), dtype=np.float32)

    def put(blob, table, name, arr):
        off, cols, rows = table[name]
        blob[:rows, off:off + cols] = arr

    put(cbA, CBA, 'w_x0cat', np.concatenate([
        inp['cnl_phi_w'].T, inp['pnl_phi_w'].T, (inp['pnl_g_w'] / M).T],
        axis=1))
    brow = np.concatenate([inp['cnl_phi_b'], inp['pnl_phi_b'],
                           inp['pnl_g_b'] / M])
    put(cbA, CBA, 'b_x0cat', np.tile(brow[None, :], (128, 1)))
    thT = inp['cnl_theta_w'].T
    put(cbA, CBA, 'w_th_bf', np.concatenate([thT[:128], thT[128:]], axis=1))
    put(cbA, CBA, 'b_th_row', inp['cnl_theta_b'][None, :])
    put(cbA, CBA, 'ones1', np.ones((1, 128), dtype=np.float32))
    put(cbA, CBA, 'ident_bf', np.eye(128, dtype=np.float32))

    th2 = inp['pnl_theta_w'].T
    put(cbB, CBB, 'w_th2', np.concatenate([th2[:128], th2[128:]], axis=1))
    w_pnlW = (scale2[:, None] * inp['pnl_W_w']).T
    put(cbB, CBB, 'w_pnlW', np.concatenate([w_pnlW, w_pnlW], axis=0))
    # sa conv banded mats; only 1/256 fold on the mean channel (no w folds)
    sa_w = np.asarray(inp['sa_conv_w'][0], dtype=np.float32).copy()
    sa_w[0] /= 256.0
    Kcat = np.zeros((2, 64, 7 * 64), dtype=np.float32)
    for ch in range(2):
        for dy in range(7):
            for dx in range(7):
                w_ = sa_w[ch, dy, dx]
                if w_ == 0.0:
                    continue
                for x in range(64):
                    xq = x + dx - 3
                    if 0 <= xq < 64:
                        Kcat[ch, xq, dy * 64 + x] = w_
    put(cbB, CBB, 'Kcat2', np.concatenate([Kcat[0], Kcat[1]], axis=1))
    Sdy = np.zeros((64, 7 * 64), dtype=np.float32)
    for dy in range(7):
        for y in range(64):
            yp = y + dy - 3
            if 0 <= yp < 64:
                Sdy[yp, dy * 64 + y] = 1.0
    perm = np.array([2 * (q % 32) + q // 32 for q in range(64)])
    put(cbB, CBB, 'Sdy7', Sdy[perm, :])
    put(cbB, CBB, 'wones', np.full((1, 128), w_fuse, dtype=np.float32))

    put(cf, CF, 'ident', np.eye(128, dtype=np.float32))
    put(cf, CF, 'w_gT', inp['cnl_g_w'] / Cl)
    bgc = (inp['cnl_g_b'] / Cl)[:, None]
    put(cf, CF, 'b_g', np.concatenate([bgc, bgc], axis=1))
    put(cf, CF, 'w_cnlW', (scale1[:, None] * inp['cnl_W_w']).T)
    put(cf, CF, 'w_tyT', (inp['pnl_theta_w'] @ (scale1[:, None] * inp['cnl_W_w'])).T)
    put(cf, CF, 'b_th2', (inp['pnl_theta_b'] + inp['pnl_theta_w'] @ cnl_bf)[:, None])
    bias2 = (pnl_bf + cnl_bf)
    put(cf, CF, 'b2', np.stack([bias2[:128], bias2[128:]], axis=1))
    fc1 = inp['ca_fc1_w'].T
    put(cf, CF, 'fc1T', np.concatenate([fc1[:128], fc1[128:]], axis=1))
    put(cf, CF, 'fc2T', inp['ca_fc2_w'].T)

    f['cbA'] = cbA.astype(ml_dtypes.bfloat16)
    f['cbB'] = cbB.astype(ml_dtypes.bfloat16)
    f['cf'] = cf.astype(np.float32)
    return f


def build_nc(w_fuse):
    nc = bacc.Bacc(None)
    x_d = nc.declare_dram_parameter("x", [128, 2, N], BF16, isOutput=False)
    x0_d = nc.declare_dram_parameter("x0", [128, N], BF16, isOutput=False)
    cbA_d = nc.declare_dram_parameter("cbA", [128, CBA_COLS], BF16, isOutput=False)
    cbB_d = nc.declare_dram_parameter("cbB", [128, CBB_COLS], BF16, isOutput=False)
    cf_d = nc.declare_dram_parameter("cf", [128, CF_COLS], F32R, isOutput=False)
    out_d = nc.declare_dram_parameter("out", [256, N], BF16, isOutput=True)
    smean_d = nc.dram_tensor("smean", [1, N], BF16)
    ssig_d = nc.dram_tensor("ssig", [1, N], BF16)

    with tile.TileContext(nc) as tc:
        _frees = []

        def _keep(pair):
            _frees.append(pair[1])
            return pair[0]

        # ---- persistent SBUF tensors ----
        x_t = _keep(tc.tile([128, 2, N], BF16, name="x_t"))
        xp_t = _keep(tc.tile([128, 2, N], BF16, name="xp_t"))
        x0_t = _keep(tc.tile([128, N], BF16, name="x0_t"))
        cbA_t = _keep(tc.tile([128, CBA_COLS], BF16, name="cbA_t"))
        cbB_t = _keep(tc.tile([128, CBB_COLS], BF16, name="cbB_t"))
        cf_t = _keep(tc.tile([128, CF_COLS], F32R, name="cf_t"))
        x0cat = _keep(tc.tile([128, 32, 256], BF16, name="x0cat"))
        thT = _keep(tc.tile([128, 32, 128], BF16, name="thT"))
        attT = _keep(tc.tile([128, 128], F32R, name="attT"))
        att_s = _keep(tc.tile([128, 128], F32R, name="att_s"))
        WA_s = _keep(tc.tile([128, 256], F32R, name="WA_s"))
        WC_s = _keep(tc.tile([128, 256], BF16, name="WC_s"))
        wta_s = _keep(tc.tile([128, 64], F32R, name="wta_s"))
        WD_s = _keep(tc.tile([128, 64], BF16, name="WD_s"))
        S2_s = _keep(tc.tile([128, 128], BF16, name="S2_s"))
        T2 = _keep(tc.tile([128, M], BF16, name="T2"))
        Y2 = _keep(tc.tile([128, M], BF16, name="Y2"))
        z_t = _keep(tc.tile([128, 2, N], BF16, name="z_t"))
        bz = _keep(tc.tile([128, 2], F32, name="bz"))
        bT2 = _keep(tc.tile([128, 1], F32, name="bT2"))
        psum_cols = _keep(tc.tile([128, 2, 4], F32, name="psum_cols"))
        macc = _keep(tc.tile([128, 2, 512], BF16, name="macc"))
        V_t = _keep(tc.tile([128, 2, 2], F32, name="V_t"))
        h_t = _keep(tc.tile([16, 2], F32, name="h_t"))
        ca_t = _keep(tc.tile([128, 2], F32, name="ca_t"))
        ca_bf = _keep(tc.tile([128, 2], BF16, name="ca_bf"))
        tmp1 = _keep(tc.tile([128, 4], F32, name="tmp1"))
        tA = _keep(tc.tile([128, N], BF16, name="tA"))
        PM = _keep(tc.tile([128, 32], BF16, name="PM"))
        PModd = _keep(tc.tile([64, 32], BF16, name="PModd"))
        m2d_sb = _keep(tc.tile([64, 64], BF16, name="m2d_sb"))
        meanrow = _keep(tc.tile([1, N], BF16, name="meanrow"))
        sigrow = _keep(tc.tile([1, N], BF16, name="sigrow"))
        mapT_mean = _keep(tc.tile([64, 64], BF16, name="mapT_mean"))
        mapT_meanP = _keep(tc.tile([64, 64], BF16, name="mapT_meanP"))
        R_sb = _keep(tc.tile([64, 448], BF16, name="R_sb"))
        sig2d = _keep(tc.tile([64, 64], BF16, name="sig2d"))
        sigb = _keep(tc.tile([128, 1, N], BF16, name="sigb"))

        def cA(name, rows=None):
            off, cols, rws = CBA[name]
            return cbA_t[0:(rows or rws), off:off + cols]

        def cB(name, rows=None):
            off, cols, rws = CBB[name]
            return cbB_t[0:(rows or rws), off:off + cols]

        def cF(name, rows=None):
            off, cols, rws = CF[name]
            return cf_t[0:(rows or rws), off:off + cols]

        from contextlib import ExitStack
        stack = ExitStack()

        # ---- DMAs: first pixel group + early consts, then the rest ----
        nc.sync.dma_start(out=x0_t[:, 0:512], in_=x0_d[:, 0:512])
        nc.sync.dma_start(out=x_t[:, :, 0:512], in_=x_d[:, :, 0:512])
        nc.sync.dma_start(out=cbA_t[:, :], in_=cbA_d[:, :])
        nc.sync.dma_start(out=x0_t[:, 512:2048], in_=x0_d[:, 512:2048])
        nc.sync.dma_start(out=x_t[:, :, 512:2048], in_=x_d[:, :, 512:2048])
        nc.sync.dma_start(out=x0_t[:, 2048:4096], in_=x0_d[:, 2048:4096])
        nc.sync.dma_start(out=x_t[:, :, 2048:4096], in_=x_d[:, :, 2048:4096])
        nc.sync.dma_start(out=cbB_t[:, :], in_=cbB_d[:, :])
        nc.sync.dma_start(out=cf_t[:, :], in_=cf_d[:, :])

        sp = stack.enter_context(tc.tile_pool(name="sp", bufs=3))

        # warm the sigmoid act-table set (contains identity/copy/relu too)
        warm = sp.tile([1, 8], F32, tag="warm", name="warm", bufs=1)
        nc.vector.memset(warm[:, :], 0.0)
        nc.scalar.activation(out=warm[:, :], in_=warm[:, :], func=AF.Sigmoid)
        onescol = sp.tile([128, 1], BF16, tag="onescol", name="onescol", bufs=1)
        nc.vector.memset(onescol[:, :], 1.0)

        # =========== Stage A: x0cat + thT + att; then S blocks ===========
        with tc.tile_pool(name="psA", bufs=2, space="PSUM") as psA, \
             tc.tile_pool(name="ps1", bufs=1, space="PSUM") as ps1:
            att_ps = ps1.tile([128, 128], F32, tag="att", name="att_ps")
            ps_s = ps1.tile([64, 256], F32, tag="S2", name="ps_s")
            for t8 in range(8):
                ps_x0c = psA.tile([128, 1024], F32, tag="x0c", name="ps_x0c")
                ps_tht = psA.tile([128, 512], F32, tag="tht", name="ps_tht")
                for sub in range(4):
                    i = 4 * t8 + sub
                    nc.tensor.matmul(ps_x0c[:, bass.ts(sub, 256)],
                                     x0_t[:, bass.ts(i, 128)], cA('w_x0cat'),
                                     start=True, stop=True)
                    nc.tensor.matmul(ps_tht[:, bass.ts(sub, 128)],
                                     cA('ones1'), cA('b_th_row'),
                                     start=True, stop=False)
                    nc.tensor.matmul(ps_tht[:, bass.ts(sub, 128)],
                                     x_t[:, 0, bass.ts(i, 128)],
                                     cA('w_th_bf')[:, 0:128],
                                     start=False, stop=False)
                    nc.tensor.matmul(ps_tht[:, bass.ts(sub, 128)],
                                     x_t[:, 1, bass.ts(i, 128)],
                                     cA('w_th_bf')[:, 128:256],
                                     start=False, stop=True)
                nc.vector.tensor_tensor(
                    out=x0cat[:, 4 * t8:4 * t8 + 4, :],
                    in0=ps_x0c[:, :].rearrange("p (a c) -> p a c", c=256),
                    in1=cA('b_x0cat').rearrange("p (a c) -> p a c", c=256
                                                ).broadcast_to([128, 4, 256]),
                    op=ALU.add)
                nc.scalar.activation(
                    out=thT[:, 4 * t8:4 * t8 + 4, :],
                    in_=ps_tht[:, :].rearrange("p (a c) -> p a c", c=128),
                    func=AF.Copy)
                for sub in range(4):
                    i = 4 * t8 + sub
                    nc.tensor.matmul(att_ps[:, :], x0cat[:, i, 0:128],
                                     thT[:, i, :], start=(i == 0), stop=(i == 31))
            # S blocks: consecutive emission (interleaving the four shared-bank
            # psum streams with other matmuls corrupts the accumulation)
            for j in range(16):
                st = (j == 0)
                sp_ = (j == 15)
                PTa = x0cat[:, j, 128:192]
                PTb = x0cat[:, j + 16, 128:192]
                GTa = x0cat[:, j, 192:256]
                GTb = x0cat[:, j + 16, 192:256]
                nc.tensor.matmul(ps_s[:, 0:64], PTa, GTa, start=st, stop=sp_)
                nc.tensor.matmul(ps_s[:, 64:128], PTa, GTb, start=st, stop=sp_)
                nc.tensor.matmul(ps_s[:, 128:192], PTb, GTa, start=st, stop=sp_)
                nc.tensor.matmul(ps_s[:, 192:256], PTb, GTb, start=st, stop=sp_)
            nc.scalar.copy(out=attT[:, :], in_=att_ps[:, :])
            nc.vector.tensor_copy(out=S2_s[0:64, :], in_=ps_s[:, 0:128])
            nc.vector.tensor_copy(out=S2_s[64:128, :], in_=ps_s[:, 128:256])

        # xp = (1-w) * x on the idle Pool engine
        for g in range(4):
            nc.gpsimd.tensor_scalar(out=xp_t[:, :, bass.ts(g, 1024)],
                                    in0=x_t[:, :, bass.ts(g, 1024)],
                                    scalar1=1.0 - w_fuse, scalar2=None,
                                    op0=ALU.mult)

        # =========== folds + T + Y + z + channel attention ===========
        with tc.tile_pool(name="psB", bufs=2, space="PSUM") as psB:
            ps_at = psB.tile([128, 128], F32R, tag="sm", name="ps_at")
            nc.tensor.transpose(_R(ps_at[:, :]), attT[:, :], _R(cF('ident')))
            nc.scalar.copy(out=att_s[:, :], in_=ps_at[:, :])
            ps_wt = psB.tile([128, 64], F32, tag="sm", name="ps_wt")
            nc.tensor.matmul(ps_wt[:, :], att_s[:, :], _R(cF('w_tyT')),
                             start=True, stop=True)
            nc.scalar.copy(out=wta_s[:, :], in_=ps_wt[:, :])
            ps_wd = psB.tile([128, 64], F32, tag="sm", name="ps_wd")
            nc.tensor.matmul(ps_wd[:, :], _R(cF('w_gT')), wta_s[:, :],
                             start=True, stop=True)
            nc.vector.tensor_copy(out=WD_s[:, :], in_=ps_wd[:, :])
            ps_bt = psB.tile([64, 2], F32, tag="sm", name="ps_bt")
            nc.tensor.matmul(ps_bt[:, :], wta_s[:, :], _R(cF('b_g')),
                             start=True, stop=True)
            nc.vector.tensor_tensor(out=bT2[0:64, :], in0=ps_bt[:, 0:1],
                                    in1=cF('b_th2').bitcast(F32), op=ALU.add)
            nc.vector.tensor_copy(out=bT2[64:128, :], in_=bT2[0:64, :])
            ps_wa = psB.tile([128, 256], F32, tag="sm", name="ps_wa")
            nc.tensor.matmul(ps_wa[:, :], att_s[:, :], _R(cF('w_cnlW')),
                             start=True, stop=True)
            nc.scalar.copy(out=WA_s[:, :], in_=ps_wa[:, :])
            ps_wc = psB.tile([128, 256], F32, tag="sm", name="ps_wc")
            nc.tensor.matmul(ps_wc[:, :], _R(cF('w_gT')), WA_s[:, :],
                             start=True, stop=True)
            nc.vector.tensor_copy(out=WC_s[:, :], in_=ps_wc[:, :])
            ps_bb = psB.tile([128, 4], F32, tag="sm", name="ps_bb")
            nc.tensor.matmul(ps_bb[:, 0:2], WA_s[:, 0:128], _R(cF('b_g')),
                             start=True, stop=True)
            nc.tensor.matmul(ps_bb[:, 2:4], WA_s[:, 128:256], _R(cF('b_g')),
                             start=True, stop=True)
            nc.vector.tensor_tensor(out=bz[:, 0:1], in0=ps_bb[:, 0:1],
                                    in1=cF('b2')[:, 0:1].bitcast(F32), op=ALU.add)
            nc.vector.tensor_tensor(out=bz[:, 1:2], in0=ps_bb[:, 2:3],
                                    in1=cF('b2')[:, 1:2].bitcast(F32), op=ALU.add)

            # ---- T2 [128, M] ----
            for tm in range(4):
                ps_T = psB.tile([128, 512], F32, tag="TY", name="ps_T")
                for h in range(2):
                    base = h * M + tm * 512
                    o = ps_T[64 * h:64 * h + 64, :]
                    nc.tensor.matmul(o, WD_s[:, :], x0_t[:, base:base + 512],
                                     start=True, stop=False)
                    nc.tensor.matmul(o, cB('w_th2')[:, 0:64],
                                     x_t[:, 0, base:base + 512],
                                     start=False, stop=False)
                    nc.tensor.matmul(o, cB('w_th2')[:, 64:128],
                                     x_t[:, 1, base:base + 512],
                                     start=False, stop=True)
                nc.scalar.activation(out=T2[:, bass.ts(tm, 512)], in_=ps_T[:, :],
                                     func=AF.Identity, bias=bT2[:, :])

            # ---- Y2 [128, M] ----
            for tm in range(4):
                ps_Y = psB.tile([128, 512], F32, tag="TY", name="ps_Y")
                nc.tensor.matmul(ps_Y[:, :], S2_s[:, :], T2[:, bass.ts(tm, 512)],
                                 start=True, stop=True)
                nc.scalar.activation(out=Y2[:, bass.ts(tm, 512)], in_=ps_Y[:, :],
                                     func=AF.Copy)

            # ---- z [128, 2, N] bf16, paired tiles per psum ----
            for t2 in range(4):
                for ch in range(2):
                    ps_z = psB.tile([128, 1024], F32, tag="z", name="ps_z")
                    for k in range(2):
                        t = 2 * t2 + k
                        h = t // 4
                        mbase = (t % 4) * 512
                        o = ps_z[:, bass.ts(k, 512)]
                        nc.tensor.matmul(o, cB('w_pnlW')[64 * h:64 * h + 64,
                                                         bass.ts(ch, 128)],
                                         Y2[64 * h:64 * h + 64, mbase:mbase + 512],
                                         start=True, stop=False)
                        nc.tensor.matmul(o, WC_s[:, bass.ts(ch, 128)],
                                         x0_t[:, bass.ts(t, 512)],
                                         start=False, stop=(ch == 1))
                        if ch == 0:
                            nc.tensor.matmul(o, cA('ident_bf'),
                                             x_t[:, 0, bass.ts(t, 512)],
                                             start=False, stop=True)
                    if ch == 0:
                        nc.scalar.activation(
                            out=z_t[:, 0, bass.ts(t2, 1024)], in_=ps_z[:, :],
                            func=AF.Identity, bias=bz[:, 0:1],
                            accum_out=psum_cols[:, 0, t2:t2 + 1])
                    else:
                        nc.vector.scalar_tensor_tensor(
                            out=z_t[:, 1, bass.ts(t2, 1024)], in0=ps_z[:, :],
                            scalar=bz[:, 1:2],
                            in1=x_t[:, 1, bass.ts(t2, 1024)], op0=ALU.add,
                            op1=ALU.add, accum_out=psum_cols[:, 1, t2:t2 + 1])
                # rolling channel-wise max accumulator (two 512-steps)
                for k in range(2):
                    t = 2 * t2 + k
                    if t == 0:
                        nc.vector.tensor_copy(out=macc[:, :, :],
                                              in_=z_t[:, :, 0:512])
                    else:
                        nc.vector.tensor_tensor(
                            out=macc[:, :, :], in0=macc[:, :, :],
                            in1=z_t[:, :, bass.ts(t, 512)], op=ALU.max)

            # ---- CBAM channel attention (compressed chain) ----
            nc.vector.reduce_max(out=tmp1[:, 0:2], in_=macc[:, :, :],
                                 axis=mybir.AxisListType.X)
            nc.scalar.activation(out=V_t[:, :, 1:2], in_=tmp1[:, 0:2],
                                 func=AF.Identity)
            nc.vector.reduce_sum(out=tmp1[:, 2:4], in_=psum_cols[:, :, :],
                                 axis=mybir.AxisListType.X)
            nc.scalar.activation(out=V_t[:, :, 0:1], in_=tmp1[:, 2:4],
                                 func=AF.Identity, scale=1.0 / float(N))
            ps_f1 = psB.tile([16, 2], F32, tag="sm", name="ps_f1")
            nc.tensor.matmul(ps_f1[:, :], cF('fc1T')[:, 0:16].bitcast(F32), V_t[:, 0, :],
                             start=True, stop=False)
            nc.tensor.matmul(ps_f1[:, :], cF('fc1T')[:, 16:32].bitcast(F32), V_t[:, 1, :],
                             start=False, stop=True)
            nc.scalar.activation(out=h_t[:, :], in_=ps_f1[:, :], func=AF.Relu)
            for ch in range(2):
                ps_f2 = psB.tile([128, 2], F32, tag="sm", name="ps_f2")
                nc.tensor.matmul(ps_f2[:, :], cF('fc2T')[:, bass.ts(ch, 128)].bitcast(F32),
                                 h_t[:, :], start=True, stop=True)
                nc.vector.reduce_sum(out=tmp1[:, ch:ch + 1], in_=ps_f2[:, :],
                                     axis=mybir.AxisListType.X)
            nc.scalar.activation(out=ca_t[:, :], in_=tmp1[:, 0:2],
                                 func=AF.Sigmoid)
            nc.vector.tensor_copy(out=ca_bf[:, :], in_=ca_t[:, :])

        # =========== maps + sa conv + final ===========
        with tc.tile_pool(name="psC", bufs=2, space="PSUM") as psC:
            # zs = z * ca in place: Act ch0; ch1 split Pool/DVE
            for g in range(2):
                nc.scalar.activation(out=z_t[:, 0, bass.ts(g, 2048)],
                                     in_=z_t[:, 0, bass.ts(g, 2048)],
                                     func=AF.Copy, scale=ca_t[:, 0:1])
            nc.gpsimd.tensor_scalar(out=z_t[:, 1, 0:2048],
                                    in0=z_t[:, 1, 0:2048],
                                    scalar1=ca_t[:, 1:2], scalar2=None,
                                    op0=ALU.mult)
            nc.vector.tensor_scalar(out=z_t[:, 1, 2048:4096],
                                    in0=z_t[:, 1, 2048:4096],
                                    scalar1=ca_t[:, 1:2], scalar2=None,
                                    op0=ALU.mult)
            # mean map from zs (ones stationary); halved DRAM roundtrip
            ps_tm = psC.tile([64, 64], BF16, tag="tm", name="ps_tm")
            for hh in range(2):
                for tq in range(4):
                    t = 4 * hh + tq
                    ps_m = psC.tile([1, 512], F32, tag="sm2", name="ps_m")
                    nc.tensor.matmul(ps_m[:, :], onescol[:, :],
                                     z_t[:, 0, bass.ts(t, 512)],
                                     start=True, stop=False)
                    nc.tensor.matmul(ps_m[:, :], onescol[:, :],
                                     z_t[:, 1, bass.ts(t, 512)],
                                     start=False, stop=True)
                    if t % 2 == 0:
                        nc.vector.tensor_copy(out=meanrow[:, bass.ts(t, 512)],
                                              in_=ps_m[:, :])
                    else:
                        nc.scalar.activation(out=meanrow[:, bass.ts(t, 512)],
                                             in_=ps_m[:, :], func=AF.Copy)
                nc.scalar.dma_start(out=smean_d[:, bass.ts(hh, 2048)],
                                    in_=meanrow[:, bass.ts(hh, 2048)])
                nc.scalar.dma_start(
                    out=m2d_sb[32 * hh:32 * hh + 32, :],
                    in_=smean_d[:, bass.ts(hh, 2048)].rearrange(
                        "p (a b) -> (p a) b", b=64))
                nc.tensor.transpose(
                    ps_tm[:, 32 * hh:32 * hh + 32],
                    m2d_sb[32 * hh:32 * hh + 32, :],
                    cA('ident_bf')[32 * hh:32 * hh + 32, 32 * hh:32 * hh + 32])
            nc.scalar.activation(
                out=mapT_meanP[:, :],
                in_=ps_tm[:, :].rearrange("p (c two) -> p two c", two=2),
                func=AF.Copy)

            # tA = max over channel chunks
            for g in range(2):
                nc.vector.tensor_tensor(out=tA[:, bass.ts(g, 2048)],
                                        in0=z_t[:, 0, bass.ts(g, 2048)],
                                        in1=z_t[:, 1, bass.ts(g, 2048)],
                                        op=ALU.max)

            # max map: transposes + per-group reduce
            for b4 in range(8):
                ps_tx = psC.tile([128, 4, 128], BF16, tag="tx", name="ps_tx")
                for k in range(4):
                    gidx = 4 * b4 + k
                    nc.tensor.transpose(ps_tx[:, k, :], tA[:, bass.ts(gidx, 128)],
                                        cA('ident_bf'))
                nc.vector.reduce_max(out=PM[:, bass.ts(b4, 4)],
                                     in_=ps_tx[:, :, :],
                                     axis=mybir.AxisListType.X)
            nc.sync.dma_start(out=PModd[:, :], in_=PM[64:128, :])

            # sa conv (banded) + sigmoid
            ps_R = psC.tile([64, 448], F32, tag="sm2", name="ps_R")
            nc.tensor.matmul(ps_R[:, :], mapT_meanP[:, :], cB('Kcat2')[:, 0:448],
                             start=True, stop=False)
            nc.tensor.matmul(ps_R[0:32, :], PM[0:64, :], cB('Kcat2')[:, 448:896],
                             start=False, stop=False)
            nc.tensor.matmul(ps_R[32:64, :], PModd[:, :], cB('Kcat2')[:, 448:896],
                             start=False, stop=True, tile_position=(0, 32))
            nc.scalar.activation(out=R_sb[:, :], in_=ps_R[:, :], func=AF.Copy)
            ps_sa = psC.tile([64, 64], F32, tag="sm2", name="ps_sa")
            for dy in range(7):
                nc.tensor.matmul(ps_sa[:, :], cB('Sdy7')[:, bass.ts(dy, 64)],
                                 R_sb[:, bass.ts(dy, 64)],
                                 start=(dy == 0), stop=(dy == 6))
            nc.scalar.activation(out=sig2d[:, :], in_=ps_sa[:, :], func=AF.Sigmoid)
            for hh in range(2):
                nc.sync.dma_start(
                    out=ssig_d[:, bass.ts(hh, 2048)].rearrange(
                        "p (a b) -> (p a) b", b=64),
                    in_=sig2d[32 * hh:32 * hh + 32, :])
                nc.sync.dma_start(out=sigrow[:, bass.ts(hh, 2048)],
                                  in_=ssig_d[:, bass.ts(hh, 2048)])

            # sig broadcast (w_fuse folded into the stationary ones)
            for t in range(8):
                ps_bc = psC.tile([128, 512], F32, tag="bc", name="ps_bc")
                nc.tensor.matmul(ps_bc[:, :], cB('wones'),
                                 sigrow[:, bass.ts(t, 512)],
                                 start=True, stop=True)
                if t % 2 == 0:
                    nc.scalar.activation(out=sigb[:, 0, bass.ts(t, 512)],
                                         in_=ps_bc[:, :], func=AF.Copy)
                else:
                    nc.vector.tensor_copy(out=sigb[:, 0, bass.ts(t, 512)],
                                          in_=ps_bc[:, :])

            # final: out = zs * sigb + xp (Pool takes group 0's multiply)
            for g in range(4):
                vt = sp.tile([128, 2, 1024], BF16, tag="vt", name="vt")
                sl = bass.ts(g, 1024)
                sgb = sigb[:, :, sl].broadcast_to([128, 2, 1024])
                eng = nc.gpsimd if g == 0 else nc.vector
                eng.tensor_tensor(out=vt[:, :, :], in0=z_t[:, :, sl], in1=sgb,
                                  op=ALU.mult)
                nc.vector.tensor_tensor(out=vt[:, :, :], in0=vt[:, :, :],
                                        in1=xp_t[:, :, sl], op=ALU.add)
                nc.sync.dma_start(
                    out=out_d[:, sl].rearrange("(two p) n -> p two n", two=2),
                    in_=vt[:, :, :])
        stack.close()
        for fr in reversed(_frees):
            fr()
    nc.compile()
    return nc


_CACHE = {}


def kernel(**inputs):
    inp = {k: np.asarray(v) for k, v in inputs.items()}
    f = fold_params(inp)
    key = round(f['w_fuse'], 9)
    if key not in _CACHE:
        _CACHE[key] = build_nc(f['w_fuse'])
    nc = _CACHE[key]

    B = inp['x'].shape[0]
    in_maps = []
    for b in range(B):
        xb = inp['x'][b].reshape(256, N).astype(np.float32)
        m = {
            'x': np.ascontiguousarray(
                xb.reshape(2, 128, N).transpose(1, 0, 2)).astype(ml_dtypes.bfloat16),
            'x0': np.ascontiguousarray(
                inp['x0'][b].reshape(128, N)).astype(ml_dtypes.bfloat16),
            'cbA': f['cbA'], 'cbB': f['cbB'], 'cf': f['cf'],
        }
        in_maps.append(m)

    res = run_bass_kernel_spmd(nc, in_maps, core_ids=list(range(B)))
    out = np.stack([np.asarray(res.results[b]['out'], dtype=np.float32
                               ).reshape(256, H, W) for b in range(B)])
    return out


# revision 26
# speedup vs baseline: 1.5063x; 1.0151x over previous
"""Trainium2 Bass kernel for nn_MDFO (CNL + PNL non-local blocks + CBAM + fusion).

Restructured v3 (pure data-parallel, B=8 over 8 cores, params replicated):
  - bf16 inputs (x, x0) uploaded from host; bf16 output, fp32 on host.
  - (1-w)*x residual computed on the otherwise-idle Pool engine.
  - all constants packed into three blob DMAs (early-bf16, late-bf16, f32).
  - y and g_x never materialized: runtime weight folds WA/WC/w_ta/WD with
    rank-1 bias fixups; T2/Y2/S2 stacked layouts halve matmul+copy counts.
  - CBAM mean via matmul accum_out, max via rolling bf16 max accumulator.
  - mean map from raw z with ca as the matmul stationary (starts at ca).
  - w_fuse folded into the sig-broadcast stationary vector.
  - final out = zs*sigb + xp with wide bf16 tensor_tensor ops.
"""
import sys

import numpy as np

sys.path.insert(0, "/opt/trn_rl_repo")

import ml_dtypes  # noqa: E402

import concourse.bass as bass  # noqa: E402
import concourse.bacc as bacc  # noqa: E402
import concourse.tile as tile  # noqa: E402
from concourse import mybir  # noqa: E402
from concourse.bass_utils import run_bass_kernel_spmd  # noqa: E402

EPS = 1e-5
F32 = mybir.dt.float32
F32R = mybir.dt.float32r
BF16 = mybir.dt.bfloat16
AF = mybir.ActivationFunctionType
ALU = mybir.AluOpType

Ch, Cl, H, W = 256, 128, 64, 64
N = H * W            # 4096
M = N // 2           # 2048
r = Cl // 2          # 64

# blob layouts: name -> (col offset, cols, rows)
CBA_COLS = 1152  # early bf16 blob
CBA = {'w_x0cat': (0, 256, 128), 'b_x0cat': (256, 256, 128),
       'w_th_bf': (512, 256, 128), 'b_th_row': (768, 128, 1),
       'ones1': (896, 128, 1), 'ident_bf': (1024, 128, 128)}
CBB_COLS = 1856  # late bf16 blob
CBB = {'w_th2': (0, 128, 128), 'w_pnlW': (128, 256, 128),
       'Kcat2': (384, 896, 64), 'Sdy7': (1280, 448, 64),
       'wones': (1728, 128, 1)}
CF_COLS = 869    # f32 blob
CF = {'ident': (0, 128, 128), 'w_gT': (128, 128, 128), 'b_g': (256, 2, 128),
      'w_cnlW': (258, 256, 128), 'w_tyT': (514, 64, 128),
      'b_th2': (578, 1, 64), 'b2': (579, 2, 128), 'fc1T': (581, 32, 128),
      'fc2T': (613, 256, 16)}


def _R(ap):
    return ap.bitcast(F32R)


def fold_params(inp):
    """Host-side constant folding into three blob arrays."""
    f = {}
    scale1 = inp['cnl_bn_g'] / np.sqrt(inp['cnl_bn_v'] + EPS)
    cnl_bf = (inp['cnl_W_b'] * scale1 + inp['cnl_bn_b']
              - inp['cnl_bn_m'] * scale1).astype(np.float32)
    scale2 = inp['pnl_bn_g'] / np.sqrt(inp['pnl_bn_v'] + EPS)
    pnl_bf = (inp['pnl_W_b'] * scale2 + inp['pnl_bn_b']
              - inp['pnl_bn_m'] * scale2).astype(np.float32)
    w_fuse = float(inp['fusion_weight'])
    f['w_fuse'] = w_fuse

    cbA = np.zeros((128, CBA_COLS), dtype=np.float32)
    cbB = np.zeros((128, CBB_COLS), dtype=np.float32)
    cf = np.zeros((128, CF_COLS), dtype=np.float32)

    def put(blob, table, name, arr):
        off, cols, rows = table[name]
        blob[:rows, off:off + cols] = arr

    put(cbA, CBA, 'w_x0cat', np.concatenate([
        inp['cnl_phi_w'].T, inp['pnl_phi_w'].T, (inp['pnl_g_w'] / M).T],
        axis=1))
    brow = np.concatenate([inp['cnl_phi_b'], inp['pnl_phi_b'],
                           inp['pnl_g_b'] / M])
    put(cbA, CBA, 'b_x0cat', np.tile(brow[None, :], (128, 1)))
    thT = inp['cnl_theta_w'].T
    put(cbA, CBA, 'w_th_bf', np.concatenate([thT[:128], thT[128:]], axis=1))
    put(cbA, CBA, 'b_th_row', inp['cnl_theta_b'][None, :])
    put(cbA, CBA, 'ones1', np.ones((1, 128), dtype=np.float32))
    put(cbA, CBA, 'ident_bf', np.eye(128, dtype=np.float32))

    th2 = inp['pnl_theta_w'].T
    put(cbB, CBB, 'w_th2', np.concatenate([th2[:128], th2[128:]], axis=1))
    w_pnlW = (scale2[:, None] * inp['pnl_W_w']).T
    put(cbB, CBB, 'w_pnlW', np.concatenate([w_pnlW, w_pnlW], axis=0))
    # sa conv banded mats; only 1/256 fold on the mean channel (no w folds)
    sa_w = np.asarray(inp['sa_conv_w'][0], dtype=np.float32).copy()
    sa_w[0] /= 256.0
    Kcat = np.zeros((2, 64, 7 * 64), dtype=np.float32)
    for ch in range(2):
        for dy in range(7):
            for dx in range(7):
                w_ = sa_w[ch, dy, dx]
                if w_ == 0.0:
                    continue
                for x in range(64):
                    xq = x + dx - 3
                    if 0 <= xq < 64:
                        Kcat[ch, xq, dy * 64 + x] = w_
    put(cbB, CBB, 'Kcat2', np.concatenate([Kcat[0], Kcat[1]], axis=1))
    Sdy = np.zeros((64, 7 * 64), dtype=np.float32)
    for dy in range(7):
        for y in range(64):
            yp = y + dy - 3
            if 0 <= yp < 64:
                Sdy[yp, dy * 64 + y] = 1.0
    perm = np.array([2 * (q % 32) + q // 32 for q in range(64)])
    put(cbB, CBB, 'Sdy7', Sdy[perm, :])
    put(cbB, CBB, 'wones', np.full((1, 128), w_fuse, dtype=np.float32))

    put(cf, CF, 'ident', np.eye(128, dtype=np.float32))
    put(cf, CF, 'w_gT', inp['cnl_g_w'] / Cl)
    bgc = (inp['cnl_g_b'] / Cl)[:, None]
    put(cf, CF, 'b_g', np.concatenate([bgc, bgc], axis=1))
    put(cf, CF, 'w_cnlW', (scale1[:, None] * inp['cnl_W_w']).T)
    put(cf, CF, 'w_tyT', (inp['pnl_theta_w'] @ (scale1[:, None] * inp['cnl_W_w'])).T)
    put(cf, CF, 'b_th2', (inp['pnl_theta_b'] + inp['pnl_theta_w'] @ cnl_bf)[:, None])
    bias2 = (pnl_bf + cnl_bf)
    put(cf, CF, 'b2', np.stack([bias2[:128], bias2[128:]], axis=1))
    fc1 = inp['ca_fc1_w'].T
    put(cf, CF, 'fc1T', np.concatenate([fc1[:128], fc1[128:]], axis=1))
    put(cf, CF, 'fc2T', inp['ca_fc2_w'].T)

    f['cbA'] = cbA.astype(ml_dtypes.bfloat16)
    f['cbB'] = cbB.astype(ml_dtypes.bfloat16)
    f['cf'] = cf.astype(np.float32)
    return f


def build_nc(w_fuse):
    nc = bacc.Bacc(None)
    x_d = nc.declare_dram_parameter("x", [128, 2, N], BF16, isOutput=False)
    x0_d = nc.declare_dram_parameter("x0", [128, N], BF16, isOutput=False)
    cbA_d = nc.declare_dram_parameter("cbA", [128, CBA_COLS], BF16, isOutput=False)
    cbB_d = nc.declare_dram_parameter("cbB", [128, CBB_COLS], BF16, isOutput=False)
    cf_d = nc.declare_dram_parameter("cf", [128, CF_COLS], F32R, isOutput=False)
    out_d = nc.declare_dram_parameter("out", [256, N], BF16, isOutput=True)
    smean_d = nc.dram_tensor("smean", [1, N], BF16)
    ssig_d = nc.dram_tensor("ssig", [1, N], BF16)

    with tile.TileContext(nc) as tc:
        _frees = []

        def _keep(pair):
            _frees.append(pair[1])
            return pair[0]

        # ---- persistent SBUF tensors ----
        x_t = _keep(tc.tile([128, 2, N], BF16, name="x_t"))
        xp_t = _keep(tc.tile([128, 2, N], BF16, name="xp_t"))
        x0_t = _keep(tc.tile([128, N], BF16, name="x0_t"))
        cbA_t = _keep(tc.tile([128, CBA_COLS], BF16, name="cbA_t"))
        cbB_t = _keep(tc.tile([128, CBB_COLS], BF16, name="cbB_t"))
        cf_t = _keep(tc.tile([128, CF_COLS], F32R, name="cf_t"))
        x0cat = _keep(tc.tile([128, 32, 256], BF16, name="x0cat"))
        thT = _keep(tc.tile([128, 32, 128], BF16, name="thT"))
        attT = _keep(tc.tile([128, 128], F32R, name="attT"))
        att_s = _keep(tc.tile([128, 128], F32R, name="att_s"))
        WA_s = _keep(tc.tile([128, 256], F32R, name="WA_s"))
        WC_s = _keep(tc.tile([128, 256], BF16, name="WC_s"))
        wta_s = _keep(tc.tile([128, 64], F32R, name="wta_s"))
        WD_s = _keep(tc.tile([128, 64], BF16, name="WD_s"))
        S2_s = _keep(tc.tile([128, 128], BF16, name="S2_s"))
        T2 = _keep(tc.tile([128, M], BF16, name="T2"))
        Y2 = _keep(tc.tile([128, M], BF16, name="Y2"))
        z_t = _keep(tc.tile([128, 2, N], BF16, name="z_t"))
        bz = _keep(tc.tile([128, 2], F32, name="bz"))
        bT2 = _keep(tc.tile([128, 1], F32, name="bT2"))
        psum_cols = _keep(tc.tile([128, 2, 4], F32, name="psum_cols"))
        macc = _keep(tc.tile([128, 2, 512], BF16, name="macc"))
        V_t = _keep(tc.tile([128, 2, 2], F32, name="V_t"))
        h_t = _keep(tc.tile([16, 2], F32, name="h_t"))
        ca_t = _keep(tc.tile([128, 2], F32, name="ca_t"))
        ca_bf = _keep(tc.tile([128, 2], BF16, name="ca_bf"))
        tmp1 = _keep(tc.tile([128, 4], F32, name="tmp1"))
        tA = _keep(tc.tile([128, N], BF16, name="tA"))
        PM = _keep(tc.tile([128, 32], BF16, name="PM"))
        PModd = _keep(tc.tile([64, 32], BF16, name="PModd"))
        m2d_sb = _keep(tc.tile([64, 64], BF16, name="m2d_sb"))
        meanrow = _keep(tc.tile([1, N], BF16, name="meanrow"))
        sigrow = _keep(tc.tile([1, N], BF16, name="sigrow"))
        mapT_mean = _keep(tc.tile([64, 64], BF16, name="mapT_mean"))
        mapT_meanP = _keep(tc.tile([64, 64], BF16, name="mapT_meanP"))
        R_sb = _keep(tc.tile([64, 448], BF16, name="R_sb"))
        sig2d = _keep(tc.tile([64, 64], BF16, name="sig2d"))
        sigb = _keep(tc.tile([128, 1, N], BF16, name="sigb"))

        def cA(name, rows=None):
            off, cols, rws = CBA[name]
            return cbA_t[0:(rows or rws), off:off + cols]

        def cB(name, rows=None):
            off, cols, rws = CBB[name]
            return cbB_t[0:(rows or rws), off:off + cols]

        def cF(name, rows=None):
            off, cols, rws = CF[name]
            return cf_t[0:(rows or rws), off:off + cols]

        from contextlib import ExitStack
        stack = ExitStack()

        # ---- DMAs: first pixel group + early consts, then the rest ----
        nc.sync.dma_start(out=x0_t[:, 0:512], in_=x0_d[:, 0:512])
        nc.sync.dma_start(out=x_t[:, :, 0:512], in_=x_d[:, :, 0:512])
        nc.sync.dma_start(out=cbA_t[:, :], in_=cbA_d[:, :])
        nc.sync.dma_start(out=x0_t[:, 512:2048], in_=x0_d[:, 512:2048])
        nc.sync.dma_start(out=x_t[:, :, 512:2048], in_=x_d[:, :, 512:2048])
        nc.sync.dma_start(out=x0_t[:, 2048:4096], in_=x0_d[:, 2048:4096])
        nc.sync.dma_start(out=x_t[:, :, 2048:4096], in_=x_d[:, :, 2048:4096])
        nc.sync.dma_start(out=cbB_t[:, :], in_=cbB_d[:, :])
        nc.sync.dma_start(out=cf_t[:, :], in_=cf_d[:, :])

        sp = stack.enter_context(tc.tile_pool(name="sp", bufs=3))

        # warm the sigmoid act-table set (contains identity/copy/relu too)
        warm = sp.tile([1, 8], F32, tag="warm", name="warm", bufs=1)
        nc.vector.memset(warm[:, :], 0.0)
        nc.scalar.activation(out=warm[:, :], in_=warm[:, :], func=AF.Sigmoid)
        onescol = sp.tile([128, 1], BF16, tag="onescol", name="onescol", bufs=1)
        nc.vector.memset(onescol[:, :], 1.0)

        # =========== Stage A: x0cat + thT + att; then S blocks ===========
        with tc.tile_pool(name="psA", bufs=2, space="PSUM") as psA, \
             tc.tile_pool(name="ps1", bufs=1, space="PSUM") as ps1:
            att_ps = ps1.tile([128, 128], F32, tag="att", name="att_ps")
            ps_s = ps1.tile([64, 256], F32, tag="S2", name="ps_s")
            for t8 in range(8):
                ps_x0c = psA.tile([128, 1024], F32, tag="x0c", name="ps_x0c")
                ps_tht = psA.tile([128, 512], F32, tag="tht", name="ps_tht")
                for sub in range(4):
                    i = 4 * t8 + sub
                    nc.tensor.matmul(ps_x0c[:, bass.ts(sub, 256)],
                                     x0_t[:, bass.ts(i, 128)], cA('w_x0cat'),
                                     start=True, stop=True)
                    nc.tensor.matmul(ps_tht[:, bass.ts(sub, 128)],
                                     cA('ones1'), cA('b_th_row'),
                                     start=True, stop=False)
                    nc.tensor.matmul(ps_tht[:, bass.ts(sub, 128)],
                                     x_t[:, 0, bass.ts(i, 128)],
                                     cA('w_th_bf')[:, 0:128],
                                     start=False, stop=False)
                    nc.tensor.matmul(ps_tht[:, bass.ts(sub, 128)],
                                     x_t[:, 1, bass.ts(i, 128)],
                                     cA('w_th_bf')[:, 128:256],
                                     start=False, stop=True)
                nc.vector.tensor_tensor(
                    out=x0cat[:, 4 * t8:4 * t8 + 4, :],
                    in0=ps_x0c[:, :].rearrange("p (a c) -> p a c", c=256),
                    in1=cA('b_x0cat').rearrange("p (a c) -> p a c", c=256
                                                ).broadcast_to([128, 4, 256]),
                    op=ALU.add)
                nc.scalar.activation(
                    out=thT[:, 4 * t8:4 * t8 + 4, :],
                    in_=ps_tht[:, :].rearrange("p (a c) -> p a c", c=128),
                    func=AF.Copy)
                for sub in range(4):
                    i = 4 * t8 + sub
                    nc.tensor.matmul(att_ps[:, :], x0cat[:, i, 0:128],
                                     thT[:, i, :], start=(i == 0), stop=(i == 31))
            # S blocks: consecutive emission (interleaving the four shared-bank
            # psum streams with other matmuls corrupts the accumulation)
            for j in range(16):
                st = (j == 0)
                sp_ = (j == 15)
                PTa = x0cat[:, j, 128:192]
                PTb = x0cat[:, j + 16, 128:192]
                GTa = x0cat[:, j, 192:256]
                GTb = x0cat[:, j + 16, 192:256]
                nc.tensor.matmul(ps_s[:, 0:64], PTa, GTa, start=st, stop=sp_)
                nc.tensor.matmul(ps_s[:, 64:128], PTa, GTb, start=st, stop=sp_)
                nc.tensor.matmul(ps_s[:, 128:192], PTb, GTa, start=st, stop=sp_)
                nc.tensor.matmul(ps_s[:, 192:256], PTb, GTb, start=st, stop=sp_)
            nc.scalar.copy(out=attT[:, :], in_=att_ps[:, :])
            nc.vector.tensor_copy(out=S2_s[0:64, :], in_=ps_s[:, 0:128])
            nc.vector.tensor_copy(out=S2_s[64:128, :], in_=ps_s[:, 128:256])

        # xp = (1-w) * x on the idle Pool engine
        for g in range(4):
            nc.gpsimd.tensor_scalar(out=xp_t[:, :, bass.ts(g, 1024)],
                                    in0=x_t[:, :, bass.ts(g, 1024)],
                                    scalar1=1.0 - w_fuse, scalar2=None,
                                    op0=ALU.mult)

        # =========== folds + T + Y + z + channel attention ===========
        with tc.tile_pool(name="psB", bufs=2, space="PSUM") as psB:
            ps_at = psB.tile([128, 128], F32R, tag="sm", name="ps_at")
            nc.tensor.transpose(_R(ps_at[:, :]), attT[:, :], _R(cF('ident')))
            nc.scalar.copy(out=att_s[:, :], in_=ps_at[:, :])
            ps_wt = psB.tile([128, 64], F32, tag="sm", name="ps_wt")
            nc.tensor.matmul(ps_wt[:, :], att_s[:, :], _R(cF('w_tyT')),
                             start=True, stop=True)
            nc.scalar.copy(out=wta_s[:, :], in_=ps_wt[:, :])
            ps_wd = psB.tile([128, 64], F32, tag="sm", name="ps_wd")
            nc.tensor.matmul(ps_wd[:, :], _R(cF('w_gT')), wta_s[:, :],
                             start=True, stop=True)
            nc.vector.tensor_copy(out=WD_s[:, :], in_=ps_wd[:, :])
            ps_bt = psB.tile([64, 2], F32, tag="sm", name="ps_bt")
            nc.tensor.matmul(ps_bt[:, :], wta_s[:, :], _R(cF('b_g')),
                             start=True, stop=True)
            nc.vector.tensor_tensor(out=bT2[0:64, :], in0=ps_bt[:, 0:1],
                                    in1=cF('b_th2').bitcast(F32), op=ALU.add)
            nc.vector.tensor_copy(out=bT2[64:128, :], in_=bT2[0:64, :])
            ps_wa = psB.tile([128, 256], F32, tag="sm", name="ps_wa")
            nc.tensor.matmul(ps_wa[:, :], att_s[:, :], _R(cF('w_cnlW')),
                             start=True, stop=True)
            nc.scalar.copy(out=WA_s[:, :], in_=ps_wa[:, :])
            ps_wc = psB.tile([128, 256], F32, tag="sm", name="ps_wc")
            nc.tensor.matmul(ps_wc[:, :], _R(cF('w_gT')), WA_s[:, :],
                             start=True, stop=True)
            nc.vector.tensor_copy(out=WC_s[:, :], in_=ps_wc[:, :])
            ps_bb = psB.tile([128, 4], F32, tag="sm", name="ps_bb")
            nc.tensor.matmul(ps_bb[:, 0:2], WA_s[:, 0:128], _R(cF('b_g')),
                             start=True, stop=True)
            nc.tensor.matmul(ps_bb[:, 2:4], WA_s[:, 128:256], _R(cF('b_g')),
                             start=True, stop=True)
            nc.vector.tensor_tensor(out=bz[:, 0:1], in0=ps_bb[:, 0:1],
                                    in1=cF('b2')[:, 0:1].bitcast(F32), op=ALU.add)
            nc.vector.tensor_tensor(out=bz[:, 1:2], in0=ps_bb[:, 2:3],
                                    in1=cF('b2')[:, 1:2].bitcast(F32), op=ALU.add)

            # ---- T2 [128, M] ----
            for tm in range(4):
                ps_T = psB.tile([128, 512], F32, tag="TY", name="ps_T")
                for h in range(2):
                    base = h * M + tm * 512
                    o = ps_T[64 * h:64 * h + 64, :]
                    nc.tensor.matmul(o, WD_s[:, :], x0_t[:, base:base + 512],
                                     start=True, stop=False)
                    nc.tensor.matmul(o, cB('w_th2')[:, 0:64],
                                     x_t[:, 0, base:base + 512],
                                     start=False, stop=False)
                    nc.tensor.matmul(o, cB('w_th2')[:, 64:128],
                                     x_t[:, 1, base:base + 512],
                                     start=False, stop=True)
                nc.scalar.activation(out=T2[:, bass.ts(tm, 512)], in_=ps_T[:, :],
                                     func=AF.Identity, bias=bT2[:, :])

            # ---- Y2 [128, M] ----
            for tm in range(4):
                ps_Y = psB.tile([128, 512], F32, tag="TY", name="ps_Y")
                nc.tensor.matmul(ps_Y[:, :], S2_s[:, :], T2[:, bass.ts(tm, 512)],
                                 start=True, stop=True)
                nc.scalar.activation(out=Y2[:, bass.ts(tm, 512)], in_=ps_Y[:, :],
                                     func=AF.Copy)

            # ---- z [128, 2, N] bf16, paired tiles per psum ----
            for t2 in range(4):
                for ch in range(2):
                    ps_z = psB.tile([128, 1024], F32, tag="z", name="ps_z")
                    for k in range(2):
                        t = 2 * t2 + k
                        h = t // 4
                        mbase = (t % 4) * 512
                        o = ps_z[:, bass.ts(k, 512)]
                        nc.tensor.matmul(o, cB('w_pnlW')[64 * h:64 * h + 64,
                                                         bass.ts(ch, 128)],
                                         Y2[64 * h:64 * h + 64, mbase:mbase + 512],
                                         start=True, stop=False)
                        nc.tensor.matmul(o, WC_s[:, bass.ts(ch, 128)],
                                         x0_t[:, bass.ts(t, 512)],
                                         start=False, stop=(ch == 1))
                        if ch == 0:
                            nc.tensor.matmul(o, cA('ident_bf'),
                                             x_t[:, 0, bass.ts(t, 512)],
                                             start=False, stop=True)
                    if ch == 0:
                        nc.scalar.activation(
                            out=z_t[:, 0, bass.ts(t2, 1024)], in_=ps_z[:, :],
                            func=AF.Identity, bias=bz[:, 0:1],
                            accum_out=psum_cols[:, 0, t2:t2 + 1])
                    else:
                        nc.vector.scalar_tensor_tensor(
                            out=z_t[:, 1, bass.ts(t2, 1024)], in0=ps_z[:, :],
                            scalar=bz[:, 1:2],
                            in1=x_t[:, 1, bass.ts(t2, 1024)], op0=ALU.add,
                            op1=ALU.add, accum_out=psum_cols[:, 1, t2:t2 + 1])
                # rolling channel-wise max accumulator (two 512-steps)
                for k in range(2):
                    t = 2 * t2 + k
                    if t == 0:
                        nc.vector.tensor_copy(out=macc[:, :, :],
                                              in_=z_t[:, :, 0:512])
                    else:
                        nc.vector.tensor_tensor(
                            out=macc[:, :, :], in0=macc[:, :, :],
                            in1=z_t[:, :, bass.ts(t, 512)], op=ALU.max)

            # ---- CBAM channel attention (compressed chain) ----
            nc.vector.reduce_max(out=tmp1[:, 0:2], in_=macc[:, :, :],
                                 axis=mybir.AxisListType.X)
            nc.scalar.activation(out=V_t[:, :, 1:2], in_=tmp1[:, 0:2],
                                 func=AF.Identity)
            nc.vector.reduce_sum(out=tmp1[:, 2:4], in_=psum_cols[:, :, :],
                                 axis=mybir.AxisListType.X)
            nc.scalar.activation(out=V_t[:, :, 0:1], in_=tmp1[:, 2:4],
                                 func=AF.Identity, scale=1.0 / float(N))
            ps_f1 = psB.tile([16, 2], F32, tag="sm", name="ps_f1")
            nc.tensor.matmul(ps_f1[:, :], cF('fc1T')[:, 0:16].bitcast(F32), V_t[:, 0, :],
                             start=True, stop=False)
            nc.tensor.matmul(ps_f1[:, :], cF('fc1T')[:, 16:32].bitcast(F32), V_t[:, 1, :],
                             start=False, stop=True)
            nc.scalar.activation(out=h_t[:, :], in_=ps_f1[:, :], func=AF.Relu)
            for ch in range(2):
                ps_f2 = psB.tile([128, 2], F32, tag="sm", name="ps_f2")
                nc.tensor.matmul(ps_f2[:, :], cF('fc2T')[:, bass.ts(ch, 128)].bitcast(F32),
                                 h_t[:, :], start=True, stop=True)
                nc.vector.reduce_sum(out=tmp1[:, ch:ch + 1], in_=ps_f2[:, :],
                                     axis=mybir.AxisListType.X)
            nc.scalar.activation(out=ca_t[:, :], in_=tmp1[:, 0:2],
                                 func=AF.Sigmoid)
            nc.vector.tensor_copy(out=ca_bf[:, :], in_=ca_t[:, :])

        # =========== maps + sa conv + final ===========
        with tc.tile_pool(name="psC", bufs=2, space="PSUM") as psC:
            # zs = z * ca in place: Act ch0; ch1 split Pool/DVE
            for g in range(2):
                nc.scalar.activation(out=z_t[:, 0, bass.ts(g, 2048)],
                                     in_=z_t[:, 0, bass.ts(g, 2048)],
                                     func=AF.Copy, scale=ca_t[:, 0:1])
            nc.gpsimd.tensor_scalar(out=z_t[:, 1, 0:2048],
                                    in0=z_t[:, 1, 0:2048],
                                    scalar1=ca_t[:, 1:2], scalar2=None,
                                    op0=ALU.mult)
            nc.vector.tensor_scalar(out=z_t[:, 1, 2048:4096],
                                    in0=z_t[:, 1, 2048:4096],
                                    scalar1=ca_t[:, 1:2], scalar2=None,
                                    op0=ALU.mult)
            # mean map from zs (ones stationary); halved DRAM roundtrip
            ps_tm = psC.tile([64, 64], BF16, tag="tm", name="ps_tm")
            for hh in range(2):
                for tq in range(4):
                    t = 4 * hh + tq
                    ps_m = psC.tile([1, 512], F32, tag="sm2", name="ps_m")
                    nc.tensor.matmul(ps_m[:, :], onescol[:, :],
                                     z_t[:, 0, bass.ts(t, 512)],
                                     start=True, stop=False)
                    nc.tensor.matmul(ps_m[:, :], onescol[:, :],
                                     z_t[:, 1, bass.ts(t, 512)],
                                     start=False, stop=True)
                    if t % 2 == 0:
                        nc.vector.tensor_copy(out=meanrow[:, bass.ts(t, 512)],
                                              in_=ps_m[:, :])
                    else:
                        nc.scalar.activation(out=meanrow[:, bass.ts(t, 512)],
                                             in_=ps_m[:, :], func=AF.Copy)
                nc.scalar.dma_start(out=smean_d[:, bass.ts(hh, 2048)],
                                    in_=meanrow[:, bass.ts(hh, 2048)])
                nc.scalar.dma_start(
                    out=m2d_sb[32 * hh:32 * hh + 32, :],
                    in_=smean_d[:, bass.ts(hh, 2048)].rearrange(
                        "p (a b) -> (p a) b", b=64))
                nc.tensor.transpose(
                    ps_tm[:, 32 * hh:32 * hh + 32],
                    m2d_sb[32 * hh:32 * hh + 32, :],
                    cA('ident_bf')[32 * hh:32 * hh + 32, 32 * hh:32 * hh + 32])
            nc.scalar.activation(
                out=mapT_meanP[:, :],
                in_=ps_tm[:, :].rearrange("p (c two) -> p two c", two=2),
                func=AF.Copy)

            # tA = max over channel chunks
            for g in range(2):
                nc.vector.tensor_tensor(out=tA[:, bass.ts(g, 2048)],
                                        in0=z_t[:, 0, bass.ts(g, 2048)],
                                        in1=z_t[:, 1, bass.ts(g, 2048)],
                                        op=ALU.max)

            # max map: transposes + per-group reduce
            for b4 in range(8):
                ps_tx = psC.tile([128, 4, 128], BF16, tag="tx", name="ps_tx")
                for k in range(4):
                    gidx = 4 * b4 + k
                    nc.tensor.transpose(ps_tx[:, k, :], tA[:, bass.ts(gidx, 128)],
                                        cA('ident_bf'))
                nc.vector.reduce_max(out=PM[:, bass.ts(b4, 4)],
                                     in_=ps_tx[:, :, :],
                                     axis=mybir.AxisListType.X)
            nc.sync.dma_start(out=PModd[:, :], in_=PM[64:128, :])

            # sa conv (banded) + sigmoid
            ps_R = psC.tile([64, 448], F32, tag="sm2", name="ps_R")
            nc.tensor.matmul(ps_R[:, :], mapT_meanP[:, :], cB('Kcat2')[:, 0:448],
                             start=True, stop=False)
            nc.tensor.matmul(ps_R[0:32, :], PM[0:64, :], cB('Kcat2')[:, 448:896],
                             start=False, stop=False)
            nc.tensor.matmul(ps_R[32:64, :], PModd[:, :], cB('Kcat2')[:, 448:896],
                             start=False, stop=True, tile_position=(0, 32))
            nc.scalar.activation(out=R_sb[:, :], in_=ps_R[:, :], func=AF.Copy)
            ps_sa = psC.tile([64, 64], F32, tag="sm2", name="ps_sa")
            for dy in range(7):
                nc.tensor.matmul(ps_sa[:, :], cB('Sdy7')[:, bass.ts(dy, 64)],
                                 R_sb[:, bass.ts(dy, 64)],
                                 start=(dy == 0), stop=(dy == 6))
            nc.scalar.activation(out=sig2d[:, :], in_=ps_sa[:, :], func=AF.Sigmoid)
            # sigrow via PE row-select matmuls (no DRAM roundtrip)
            for t in range(8):
                ps_sg = psC.tile([1, 512], F32, tag="sm2", name="ps_sg")
                for k in range(8):
                    y = 8 * t + k
                    nc.tensor.matmul(ps_sg[0:1, bass.ts(k, 64)],
                                     cA('ident_bf')[0:64, y:y + 1],
                                     sig2d[:, :], start=True, stop=True)
                if t % 2 == 0:
                    nc.vector.tensor_copy(out=sigrow[:, bass.ts(t, 512)],
                                          in_=ps_sg[:, :])
                else:
                    nc.scalar.activation(out=sigrow[:, bass.ts(t, 512)],
                                         in_=ps_sg[:, :], func=AF.Copy)

            # sig broadcast (w_fuse folded into the stationary ones)
            for t in range(8):
                ps_bc = psC.tile([128, 512], F32, tag="bc", name="ps_bc")
                nc.tensor.matmul(ps_bc[:, :], cB('wones'),
                                 sigrow[:, bass.ts(t, 512)],
                                 start=True, stop=True)
                if t % 2 == 0:
                    nc.scalar.activation(out=sigb[:, 0, bass.ts(t, 512)],
                                         in_=ps_bc[:, :], func=AF.Copy)
                else:
                    nc.vector.tensor_copy(out=sigb[:, 0, bass.ts(t, 512)],
                                          in_=ps_bc[:, :])

            # final: out = zs * sigb + xp (Pool takes group 0's multiply)
            for g in range(4):
                vt = sp.tile([128, 2, 1024], BF16, tag="vt", name="vt")
                sl = bass.ts(g, 1024)
                sgb = sigb[:, :, sl].broadcast_to([128, 2, 1024])
                eng = nc.gpsimd if g == 0 else nc.vector
                eng.tensor_tensor(out=vt[:, :, :], in0=z_t[:, :, sl], in1=sgb,
                                  op=ALU.mult)
                nc.vector.tensor_tensor(out=vt[:, :, :], in0=vt[:, :, :],
                                        in1=xp_t[:, :, sl], op=ALU.add)
                nc.sync.dma_start(
                    out=out_d[:, sl].rearrange("(two p) n -> p two n", two=2),
                    in_=vt[:, :, :])
        stack.close()
        for fr in reversed(_frees):
            fr()
    nc.compile()
    return nc


_CACHE = {}


def kernel(**inputs):
    inp = {k: np.asarray(v) for k, v in inputs.items()}
    f = fold_params(inp)
    key = round(f['w_fuse'], 9)
    if key not in _CACHE:
        _CACHE[key] = build_nc(f['w_fuse'])
    nc = _CACHE[key]

    B = inp['x'].shape[0]
    in_maps = []
    for b in range(B):
        xb = inp['x'][b].reshape(256, N).astype(np.float32)
        m = {
            'x': np.ascontiguousarray(
                xb.reshape(2, 128, N).transpose(1, 0, 2)).astype(ml_dtypes.bfloat16),
            'x0': np.ascontiguousarray(
                inp['x0'][b].reshape(128, N)).astype(ml_dtypes.bfloat16),
            'cbA': f['cbA'], 'cbB': f['cbB'], 'cf': f['cf'],
        }
        in_maps.append(m)

    res = run_bass_kernel_spmd(nc, in_maps, core_ids=list(range(B)))
    out = np.stack([np.asarray(res.results[b]['out'], dtype=np.float32
                               ).reshape(256, H, W) for b in range(B)])
    return out
